# revision 1
# baseline (speedup 1.0000x reference)
"""Entmax attention Trainium2 kernel v2 (8-core SPMD, head-parallel).

Math (matches reference _entmax_naive):
  scores = (q*scale) @ k^T  (fp16 hi/lo 3-term matmul, ~2^-21 accurate)
  per row: Newton x8 from t0 = rowmax-1 on fp32 scores for tau* root of
           sum relu(s-t) = 1; k = #{s > t_final}; tau_star = (rowsum-1)/k
  Z = sum relu(s - tau_star)  (one more F pass, fp32)
  P~^T produced directly transposed via a second PE matmul:
      S^T_ext[j,i] = sum_d kh[d,j]*(qh[d,i]*rz_i) + 1*(-tau_star_i*rz_i)
                   = rz_i*(s_ij - tau_star_i);   P~^T = relu(S^T_ext)
      (rz = 1/(Z+1e-10); relu(x)/Z = relu(x/Z) makes P~ pre-normalized,
       so AV and Wo need no further normalization)
  out^T = V^T-stationary AV matmuls consuming P~^T; Wo consumes out^T.

Engine plan: Newton F on ACT (Relu+bias accum) for half the units and on
DVE (scalar_tensor_tensor max accum) for the other half; counts via DVE
is_gt and ACT Sign trick. All iteration passes fp32 (ACT/DVE accumulate
ops run 1x regardless of dtype, so bf16 buys nothing).

Sharding: 16 heads / 8 cores = 2 heads per core; host sums Wo partials.
"""
import numpy as np
from contextlib import ExitStack

import concourse.bass as bass
import concourse.tile as tile
import concourse.mybir as mybir
from concourse import bacc
from concourse.bass_utils import run_bass_kernel_spmd

L = 2048
D = 1024
H = 16
HD = 64
N_CORES = 8
HPC = 2
SCALE = float(HD) ** -0.5

FP32 = mybir.dt.float32
FP16 = mybir.dt.float16
BF16 = mybir.dt.bfloat16
Alu = mybir.AluOpType
Act = mybir.ActivationFunctionType

N_ITERS = 8
N_SEC = 10
NEG_BIG = -1.0e30
MAX_INIT = -3.0e38

RB_PAIRS = [(0, 15), (4, 11), (1, 14), (5, 10), (2, 13), (6, 9), (3, 12), (7, 8)]
# jt bins for packed P^T slots: each bin's widths (16-jt) sum to 34 blocks
PT_BINS = [[0, 3, 12, 15], [1, 4, 11, 14], [2, 5, 10, 13], [6, 7, 8, 9]]


def _units_of_group(g):
    """8 units: (rb, head, slot, col_off); stat col = 8g+ui, h-major so the
    (g,h) 4-col slices are contiguous."""
    units = []
    p0, p1 = RB_PAIRS[2 * g], RB_PAIRS[2 * g + 1]
    for h in range(HPC):
        for pi_local, (ra, rb_) in enumerate((p0, p1)):
            slot = 2 * pi_local + h
            na = 128 * (ra + 1)
            units.append((ra, h, slot, 0))
            units.append((rb_, h, slot, na))
    return units


def build_program(debug_out=None):
    nc = bacc.Bacc("TRN2", target_bir_lowering=False, debug=False, num_devices=1)

    xT_d = nc.dram_tensor("xT", [D, L], FP32, kind="ExternalInput")
    wq_d = nc.dram_tensor("wqT", [D, 128], FP32, kind="ExternalInput")
    wk_d = nc.dram_tensor("wkT", [D, 128], FP32, kind="ExternalInput")
    wv_d = nc.dram_tensor("wvT", [D, 128], FP32, kind="ExternalInput")
    wo_d = nc.dram_tensor("woT", [128, D], FP32, kind="ExternalInput")
    mneg_d = nc.dram_tensor("mneg", [128, 128], FP32, kind="ExternalInput")
    m01_d = nc.dram_tensor("m01", [128, 128], FP32, kind="ExternalInput")
    m01u_d = nc.dram_tensor("m01u", [128, 128], FP16, kind="ExternalInput")
    ident_d = nc.dram_tensor("ident", [128, 128], FP32, kind="ExternalInput")
    out_d = nc.dram_tensor("out", [L, D], FP32, kind="ExternalOutput")

    PAIR_W = 17 * 128  # 2176

    with tile.TileContext(nc) as tc:
        with ExitStack() as ctx:
            persist = ctx.enter_context(tc.tile_pool(name="persist", bufs=1))
            # fp16 q/k hi parts per head with a 65th row; lo parts packed
            qhx = [persist.tile([65, L], FP16, tag=f"qhx{h}", name=f"qhx{h}") for h in range(2)]
            khx = [persist.tile([65, L], FP16, tag=f"khx{h}", name=f"khx{h}") for h in range(2)]
            qtl = [persist.tile([64, L], FP16, tag=f"qtl{h}", name=f"qtl{h}")
                   for h in range(2)]
            ktl = [persist.tile([64, L], FP16, tag=f"ktl{h}", name=f"ktl{h}")
                   for h in range(2)]
            vt = persist.tile([128, 16, 130], FP16, tag="vt", name="vt")  # [j, jt, d(2 heads)]
            woTh = persist.tile([128, D], FP16, tag="woTh", name="woTh")
            outTh = persist.tile([128, L], FP16, tag="outTh", name="outTh")  # [d(2h), i]
            mneg = persist.tile([128, 128], FP32, tag="mneg", name="mneg")
            m01 = persist.tile([128, 128], FP32, tag="m01", name="m01")
            m01u = persist.tile([128, 128], FP16, tag="m01u", name="m01u")
            ident = persist.tile([128, 128], FP32, tag="ident", name="ident")

            zeros_bf = persist.tile([128, L], BF16, tag="zbf", name="zbf")
            zsb = [persist.tile([1, L], FP32, tag=f"zsb{h}", name=f"zsb{h}")
                   for h in range(2)]
            trash_a = persist.tile([128, L], BF16, tag="tra", name="tra")
            trash_d = persist.tile([128, L], BF16, tag="trd", name="trd")
            trash_c = persist.tile([128, L], BF16, tag="trc", name="trc")

            NST = 32

            def stat(tag):
                return persist.tile([128, NST], FP32, tag=tag, name=tag)

            maxF, maxD, maxG = stat("maxF"), stat("maxD"), stat("maxG")
            sumF, sumD, sumG = stat("sumF"), stat("sumD"), stat("sumG")
            mx, sm = stat("mx"), stat("sm")
            Tt, nT = stat("T"), stat("nT")
            Ft, Ct = stat("F"), stat("C")
            rec, Fm, dlt = stat("rec"), stat("Fm"), stat("dlt")
            tp_t, Fp_t = stat("tp"), stat("Fp")
            tau, ntau = stat("tau"), stat("ntau")
            rzt = stat("rzt")
            nh = stat("nh")

            nc.sync.dma_start(mneg[:], mneg_d.ap())
            nc.sync.dma_start(m01[:], m01_d.ap())
            nc.sync.dma_start(m01u[:], m01u_d.ap())
            nc.sync.dma_start(ident[:], ident_d.ap())
            nc.vector.memset(zeros_bf[:], 0.0)
            nc.vector.memset(maxF[:], MAX_INIT)
            nc.vector.memset(sumF[:], 0.0)
            nc.vector.memset(maxD[:], MAX_INIT)
            nc.vector.memset(sumD[:], 0.0)
            nc.vector.memset(maxG[:], MAX_INIT)
            nc.vector.memset(sumG[:], 0.0)
            for h in range(2):
                nc.vector.memset(khx[h][64:65, :], 1.0)
                nc.vector.memset(qhx[h][64:65, :], 0.0)
                nc.vector.memset(vt[:, :, 65 * h + 64], 1.0)
            for g in range(4):
                for ui, (rb, h, slot, off) in enumerate(_units_of_group(g)):
                    col = 8 * g + ui
                    nc.vector.memset(nh[:, col:col + 1], 64.0 * (rb + 1))

            # ---------- phase 1: projections ----------
            with ExitStack() as p1:
                ph1 = p1.enter_context(tc.tile_pool(name="ph1", bufs=1))
                ph1p = p1.enter_context(
                    tc.tile_pool(name="ph1p", bufs=2, space="PSUM"))
                xt = ph1.tile([128, 8, L], FP32, tag="xt", name="xt")
                wqs = ph1.tile([128, 8, 128], FP32, tag="wqs", name="wqs")
                wks = ph1.tile([128, 8, 128], FP32, tag="wks", name="wks")
                wvs = ph1.tile([128, 8, 128], FP32, tag="wvs", name="wvs")

                xview = xT_d.ap().rearrange("(c p) n -> p c n", p=128)
                for c in range(8):
                    nc.sync.dma_start(xt[:, c, :], xview[:, c, :])
                nc.sync.dma_start(wqs[:], wq_d.ap().rearrange("(c p) m -> p c m", p=128))
                nc.sync.dma_start(wks[:], wk_d.ap().rearrange("(c p) m -> p c m", p=128))
                nc.sync.dma_start(wvs[:], wv_d.ap().rearrange("(c p) m -> p c m", p=128))
                wo32 = ph1.tile([128, D], FP32, tag="wo32", name="wo32")
                nc.sync.dma_start(wo32[:], wo_d.ap())
                nc.scalar.copy(woTh[:], wo32[:])

                for which, wsb in ((0, wqs), (1, wks)):
                    hx = qhx if which == 0 else khx
                    lo = qtl if which == 0 else ktl
                    for ic in range(4):
                        ps = ph1p.tile([128, 512], FP32, tag="pp")
                        for e in range(8):
                            nc.tensor.matmul(
                                ps[:], wsb[:, e, :], xt[:, e, 512 * ic:512 * (ic + 1)],
                                start=(e == 0), stop=(e == 7))
                        cs = slice(512 * ic, 512 * (ic + 1))
                        for h in range(2):
                            rs = slice(64 * h, 64 * h + 64)
                            nc.scalar.copy(hx[h][0:64, cs], ps[rs, :])
                            nc.vector.tensor_tensor(
                                lo[h][:, cs], ps[rs, :], hx[h][0:64, cs],
                                Alu.subtract)
                for jt in range(16):
                    ps = ph1p.tile([128, 512], FP32, tag="pp")
                    for e in range(8):
                        nc.tensor.matmul(
                            ps[:, :128], xt[:, e, 128 * jt:128 * (jt + 1)], wvs[:, e, :],
                            start=(e == 0), stop=(e == 7))
                    if jt % 2 == 0:
                        nc.scalar.copy(vt[:, jt, 0:64], ps[:, 0:64])
                        nc.vector.tensor_copy(vt[:, jt, 65:129], ps[:, 64:128])
                    else:
                        nc.vector.tensor_copy(vt[:, jt, 0:64], ps[:, 0:64])
                        nc.scalar.copy(vt[:, jt, 65:129], ps[:, 64:128])

            if debug_out == "qkv":
                flat = out_d.ap().rearrange("a b -> (a b)")
                nc.sync.dma_start(flat[0:133120], qhx[0][:])
                nc.sync.dma_start(flat[133120:266240], khx[0][:])

            # ---------- phase 2 pools ----------
            s_pool = ctx.enter_context(tc.tile_pool(name="spair", bufs=2))
            xf_pool = ctx.enter_context(tc.tile_pool(name="xf", bufs=2))
            stage_pool = ctx.enter_context(tc.tile_pool(name="stg", bufs=2))
            p2 = ExitStack()
            ps_sc = p2.enter_context(tc.tile_pool(name="ps_sc", bufs=3, space="PSUM"))
            ps_rz = p2.enter_context(tc.tile_pool(name="ps_rz", bufs=1, space="PSUM"))

            copy_flip = [0]

            def balanced_copy(dst, src):
                if copy_flip[0] % 2 == 0:
                    nc.scalar.copy(dst, src)
                else:
                    nc.vector.tensor_copy(dst, src)
                copy_flip[0] += 1

            all_S = {}

            for g in range(4):
                units = _units_of_group(g)
                gsl = slice(8 * g, 8 * g + 8)
                hsl = slice(8 * g + 4, 8 * g + 8)
                Sg = [s_pool.tile([128, PAIR_W], FP32, tag=f"sp{s}", name=f"sp{s}_{g}")
                      for s in range(4)]
                all_S[g] = Sg

                # ---- A/B: fp16 3-term scores -> psum; evict + stats ----
                for ui, (rb, h, slot, off) in enumerate(units):
                    col = 8 * g + ui
                    n = 128 * (rb + 1)
                    S = Sg[slot]
                    rbs = slice(128 * rb, 128 * rb + 128)
                    for ci, c0 in enumerate(range(0, n, 1024)):
                        w = min(1024, n - c0)
                        ps = ps_sc.tile([128, 1024], FP32, tag="sc",
                                        name=f"sc{g}_{ui}_{ci}")
                        for s0 in range(0, w, 512):
                            sw = min(512, w - s0)
                            cs = slice(c0 + s0, c0 + s0 + sw)
                            pss = ps[:, s0:s0 + sw]
                            nc.tensor.matmul(pss, qhx[h][0:64, rbs],
                                             khx[h][0:64, cs],
                                             start=True, stop=False)
                            nc.tensor.matmul(pss, qhx[h][0:64, rbs],
                                             ktl[h][:, cs],
                                             start=False, stop=False)
                            nc.tensor.matmul(pss, qtl[h][:, rbs],
                                             khx[h][0:64, cs],
                                             start=False, stop=True)
                        last = (c0 + w == n)
                        mainw = w - 128 if last else w
                        s_acc = sumF if ci == 0 else sumG
                        m_acc = maxF if ci == 0 else maxG
                        if mainw > 0:
                            nc.scalar.activation(
                                trash_a[:, :mainw], ps[:, :mainw], Act.Identity,
                                bias=0.0, accum_out=s_acc[:, col:col + 1])
                            nc.vector.tensor_scalar(
                                out=S[:, off + c0:off + c0 + mainw],
                                in0=ps[:, :mainw],
                                scalar1=0.0, scalar2=MAX_INIT,
                                op0=Alu.add, op1=Alu.max,
                                accum_out=m_acc[:, col:col + 1])
                        if last:
                            nc.vector.tensor_tensor(
                                S[:, off + n - 128:off + n],
                                ps[:, mainw:w], mneg[:], Alu.add)
                            nc.vector.tensor_scalar(
                                out=trash_c[:, :128],
                                in0=S[:, off + n - 128:off + n],
                                scalar1=0.0, scalar2=MAX_INIT,
                                op0=Alu.add, op1=Alu.max,
                                accum_out=maxD[:, col:col + 1])
                            nc.vector.scalar_tensor_tensor(
                                out=trash_d[:, :128],
                                in0=ps[:, mainw:w], scalar=1.0, in1=m01[:],
                                op0=Alu.mult, op1=Alu.mult,
                                accum_out=sumD[:, col:col + 1])

                nc.vector.tensor_tensor(mx[:, gsl], maxF[:, gsl], maxD[:, gsl], Alu.max)
                nc.vector.tensor_tensor(mx[:, gsl], mx[:, gsl], maxG[:, gsl], Alu.max)
                nc.vector.tensor_tensor(sm[:, gsl], sumF[:, gsl], sumD[:, gsl], Alu.add)
                nc.vector.tensor_tensor(sm[:, gsl], sm[:, gsl], sumG[:, gsl], Alu.add)
                nc.vector.tensor_scalar_add(Tt[:, gsl], mx[:, gsl], -1.0)
                nc.vector.tensor_scalar(
                    out=nT[:, gsl], in0=mx[:, gsl], scalar1=-1.0, scalar2=1.0,
                    op0=Alu.mult, op1=Alu.add)

                if debug_out == "scores":
                    flat2 = out_d.ap().rearrange("a b -> (a b)")
                    for slot in range(4):
                        nc.sync.dma_start(
                            flat2[278528 * slot:278528 * (slot + 1)], Sg[slot][:])
                    continue

                # ---- Newton iterations (fp32) ----
                def emit_passes(skip_F=False, skip_C=False):
                    for ui, (rb, h, slot, off) in enumerate(units):
                        col = 8 * g + ui
                        n = 128 * (rb + 1)
                        Ssrc = Sg[slot]
                        if ui < 4:
                            if not skip_F:
                                nc.scalar.activation(
                                    trash_a[:, :n], Ssrc[:, off:off + n], Act.Relu,
                                    bias=nT[:, col:col + 1],
                                    accum_out=Ft[:, col:col + 1])
                            if not skip_C:
                                nc.vector.tensor_scalar(
                                    out=trash_c[:, :n], in0=Ssrc[:, off:off + n],
                                    scalar1=Tt[:, col:col + 1], scalar2=0.0,
                                    op0=Alu.is_gt, op1=Alu.add,
                                    accum_out=Ct[:, col:col + 1])
                        else:
                            if not skip_F:
                                nc.vector.scalar_tensor_tensor(
                                    out=trash_d[:, :n], in0=Ssrc[:, off:off + n],
                                    scalar=nT[:, col:col + 1], in1=zeros_bf[:, :n],
                                    op0=Alu.add, op1=Alu.max,
                                    accum_out=Ft[:, col:col + 1])
                            if not skip_C:
                                nc.scalar.activation(
                                    trash_a[:, :n], Ssrc[:, off:off + n], Act.Sign,
                                    bias=nT[:, col:col + 1],
                                    accum_out=Ct[:, col:col + 1])
                    if not skip_C:
                        nc.vector.scalar_tensor_tensor(
                            out=Ct[:, hsl], in0=Ct[:, hsl], scalar=0.5, in1=nh[:, hsl],
                            op0=Alu.mult, op1=Alu.add)

                def newton_update():
                    nc.vector.tensor_scalar_max(Ct[:, gsl], Ct[:, gsl], 1.0)
                    nc.vector.reciprocal(rec[:, gsl], Ct[:, gsl])
                    nc.vector.tensor_scalar_add(Fm[:, gsl], Ft[:, gsl], -1.0)
                    nc.vector.tensor_tensor(dlt[:, gsl], Fm[:, gsl], rec[:, gsl], Alu.mult)
                    nc.vector.tensor_tensor(Tt[:, gsl], Tt[:, gsl], dlt[:, gsl], Alu.add)
                    nc.vector.tensor_tensor(nT[:, gsl], nT[:, gsl], dlt[:, gsl], Alu.subtract)

                # iteration 1: full Newton (F+C); then F-only secant
                emit_passes()
                nc.vector.tensor_copy(tp_t[:, gsl], Tt[:, gsl])
                nc.vector.tensor_copy(Fp_t[:, gsl], Ft[:, gsl])
                newton_update()
                for _ in range(N_SEC):
                    emit_passes(skip_C=True)
                    nc.vector.tensor_tensor(
                        dlt[:, gsl], Tt[:, gsl], tp_t[:, gsl], Alu.subtract)
                    nc.vector.tensor_tensor(
                        rec[:, gsl], Fp_t[:, gsl], Ft[:, gsl], Alu.subtract)
                    nc.vector.tensor_scalar_max(rec[:, gsl], rec[:, gsl], 1.0e-20)
                    nc.vector.reciprocal(rec[:, gsl], rec[:, gsl])
                    nc.vector.scalar_tensor_tensor(
                        out=Fm[:, gsl], in0=Ft[:, gsl], scalar=-1.0,
                        in1=dlt[:, gsl], op0=Alu.add, op1=Alu.mult)
                    nc.vector.tensor_copy(tp_t[:, gsl], Tt[:, gsl])
                    nc.vector.tensor_copy(Fp_t[:, gsl], Ft[:, gsl])
                    nc.vector.tensor_tensor(
                        dlt[:, gsl], Fm[:, gsl], rec[:, gsl], Alu.mult)
                    nc.vector.tensor_scalar(
                        out=dlt[:, gsl], in0=dlt[:, gsl], scalar1=0.0,
                        scalar2=1.0, op0=Alu.max, op1=Alu.min)
                    nc.vector.tensor_tensor(
                        Tt[:, gsl], Tt[:, gsl], dlt[:, gsl], Alu.add)
                    nc.vector.tensor_tensor(
                        nT[:, gsl], nT[:, gsl], dlt[:, gsl], Alu.subtract)

                # ---- final count -> k; tau_star; Z pass; rz/ntr xfer ----
                emit_passes(skip_F=True)
                nc.vector.tensor_scalar_max(Ct[:, gsl], Ct[:, gsl], 1.0)
                nc.vector.reciprocal(rec[:, gsl], Ct[:, gsl])
                nc.vector.tensor_scalar_add(Fm[:, gsl], sm[:, gsl], -1.0)
                nc.vector.tensor_tensor(tau[:, gsl], Fm[:, gsl], rec[:, gsl], Alu.mult)
                nc.vector.tensor_scalar_mul(ntau[:, gsl], tau[:, gsl], -1.0)


                if debug_out == "tau":
                    flat3 = out_d.ap().rearrange("a b -> (a b)")
                    nc.sync.dma_start(flat3[4096 * g:4096 * g + 2048],
                                      tau[:, gsl].rearrange("p c -> (c p)"))
                    nc.sync.dma_start(flat3[4096 * g + 2048:4096 * (g + 1)],
                                      ntau[:, gsl].rearrange("p c -> (c p)"))
                    continue

                # ---- per (g,h): route -tau_star into qhx row 64 ----
                for h in range(2):
                    hcols = slice(8 * g + 4 * h, 8 * g + 4 * h + 4)
                    rbs_list = [units[4 * h + i][0] for i in range(4)]
                    xf = xf_pool.tile([128, 4], FP32, tag=f"xf{h}", name=f"xf{g}_{h}")
                    nc.vector.tensor_copy(xf[:], ntau[:, hcols])
                    psx = ps_rz.tile([4, 128], FP32, tag="psx", name=f"psx{g}_{h}")
                    nc.tensor.transpose(psx[:], xf[:], ident[:])
                    stg = stage_pool.tile([4, 128], FP32, tag=f"stg{h}",
                                          name=f"stg{g}_{h}")
                    nc.scalar.copy(stg[:], psx[:])
                    p64 = ps_rz.tile([1, 512], FP32, tag="p64", name=f"p64{g}_{h}")
                    for b in range(4):
                        nc.tensor.matmul(
                            p64[:, 128 * b:128 * (b + 1)],
                            ident[0:4, b:b + 1], stg[:], start=True, stop=True)
                    for b in range(4):
                        rb = rbs_list[b]
                        osl = slice(128 * rb, 128 * rb + 128)
                        nc.scalar.copy(qhx[h][64:65, osl],
                                       p64[:, 128 * b:128 * (b + 1)])

            p2.close()

            if debug_out not in ("scores", "tau"):
                with ExitStack() as tl:
                    ps_st = tl.enter_context(
                        tc.tile_pool(name="ps_st", bufs=2, space="PSUM"))
                    ps_av = tl.enter_context(
                        tc.tile_pool(name="ps_av", bufs=1, space="PSUM"))
                    ps_wo = tl.enter_context(
                        tc.tile_pool(name="ps_wo", bufs=1, space="PSUM"))
                    ps_z = tl.enter_context(
                        tc.tile_pool(name="ps_z", bufs=1, space="PSUM"))
                    wo_pool = tl.enter_context(tc.tile_pool(name="woout", bufs=2))

                    # P^T slots reuse the S pool buffers: 4 bins x 2 heads,
                    # each [128, 4352] fp16 (= S slot bytes); bin b holds
                    # jts PT_BINS[b].
                    PT = {}
                    pt_off = {}
                    for b, jts in enumerate(PT_BINS):
                        off = 0
                        for jt in jts:
                            pt_off[jt] = (b, off)
                            off += (16 - jt) * 128
                    for h in range(2):
                        for b in range(4):
                            PT[(h, b)] = s_pool.tile(
                                [128, 4352], FP16, tag=f"sp{b}", name=f"pt{h}_{b}")

                    # S^T + relu eviction -> P~^T
                    for h in range(2):
                        for jt in range(16):
                            w = (16 - jt) * 128
                            b, off = pt_off[jt]
                            pt_tile = PT[(h, b)]
                            jsl = slice(128 * jt, 128 * jt + 128)
                            for c0 in range(0, w, 512):
                                cw = min(512, w - c0)
                                ps = ps_st.tile([128, 512], FP32, tag="st",
                                                name=f"st{h}_{jt}_{c0}")
                                isl = slice(128 * jt + c0, 128 * jt + c0 + cw)
                                nc.tensor.matmul(
                                    ps[:, :cw], khx[h][:, jsl], qhx[h][:, isl],
                                    start=True, stop=True)
                                d0 = 128 if c0 == 0 else 0
                                if d0:
                                    nc.vector.scalar_tensor_tensor(
                                        out=pt_tile[:, off + c0:off + c0 + 128],
                                        in0=ps[:, 0:128], scalar=0.0, in1=m01u[:],
                                        op0=Alu.max, op1=Alu.mult)
                                if cw > d0:
                                    if (jt + h) % 2 == 0:
                                        nc.scalar.activation(
                                            pt_tile[:, off + c0 + d0:off + c0 + cw],
                                            ps[:, d0:cw], Act.Relu, bias=0.0)
                                    else:
                                        nc.vector.tensor_scalar(
                                            out=pt_tile[:, off + c0 + d0:off + c0 + cw],
                                            in0=ps[:, d0:cw], scalar1=0.0,
                                            scalar2=0.0, op0=Alu.max, op1=Alu.add)

                    # AV: out^T[d, i] per (h, half): accumulate over jt
                    for h in range(2):
                        for half in range(2):
                            avp = ps_av.tile([65, 1024], FP32, tag="av",
                                             name=f"av{h}_{half}")
                            first = True
                            for jt in (range(0, 8) if half == 0 else range(0, 16)):
                                b, off = pt_off[jt]
                                tile_i0 = 128 * jt
                                lo = max(1024 * half, tile_i0)
                                hi = 1024 * (half + 1)
                                if lo >= hi:
                                    continue
                                for cc in range(lo, hi, 512):
                                    ce = min(cc + 512, hi)
                                    src = PT[(h, b)][:, off + (cc - tile_i0):
                                                     off + (ce - tile_i0)]
                                    nc.tensor.matmul(
                                        avp[:, cc - 1024 * half:ce - 1024 * half],
                                        vt[:, jt, 65 * h:65 * h + 65], src,
                                        start=first,
                                        stop=(jt == (7 if half == 0 else 15)))
                                first = False
                            balanced_copy(
                                outTh[64 * h:64 * h + 64,
                                      1024 * half:1024 * (half + 1)], avp[0:64, :])
                            nc.scalar.copy(
                                zsb[h][0:1, 1024 * half:1024 * (half + 1)],
                                avp[64:65, :])

                    # Z row -> stats layout via tiny PE transposes; rz2
                    zps = ps_z.tile([128, 32], FP32, tag="zps", name="zps")
                    for h in range(2):
                        for b in range(16):
                            nc.tensor.transpose(
                                zps[:, 16 * h + b:16 * h + b + 1],
                                zsb[h][0:1, 128 * b:128 * b + 128],
                                ident[0:1, 0:1])
                    nc.vector.tensor_scalar_add(rzt[:], zps[:], 1.0e-10)
                    nc.vector.reciprocal(rzt[:], rzt[:])

                    # Wo per i-block, per head, normalized at eviction
                    for blk in range(16):
                        wo_sb = wo_pool.tile([128, D], FP32, tag="wod",
                                             name=f"wod{blk}")
                        for oc in range(2):
                            osl2 = slice(512 * oc, 512 * (oc + 1))
                            wop0 = ps_wo.tile([128, 512], FP32, tag="wo0",
                                              name=f"wo0_{blk}_{oc}")
                            wop1 = ps_wo.tile([128, 512], FP32, tag="wo1",
                                              name=f"wo1_{blk}_{oc}")
                            nc.tensor.matmul(
                                wop0[:], outTh[0:64, 128 * blk:128 * (blk + 1)],
                                woTh[0:64, osl2], start=True, stop=True)
                            nc.tensor.matmul(
                                wop1[:], outTh[64:128, 128 * blk:128 * (blk + 1)],
                                woTh[64:128, osl2], start=True, stop=True)
                            nc.scalar.activation(
                                wo_sb[:, osl2], wop0[:], Act.Copy,
                                bias=0.0, scale=rzt[:, blk:blk + 1])
                            nc.vector.scalar_tensor_tensor(
                                out=wo_sb[:, osl2], in0=wop1[:],
                                scalar=rzt[:, 16 + blk:16 + blk + 1],
                                in1=wo_sb[:, osl2], op0=Alu.mult, op1=Alu.add)
                        nc.sync.dma_start(
                            out_d.ap()[128 * blk:128 * (blk + 1), :], wo_sb[:])

    nc.compile()
    return nc



_CACHE = {}


def _get_nc(debug_out=None):
    key = f"nc_{debug_out}"
    if key not in _CACHE:
        _CACHE[key] = build_program(debug_out)
    return _CACHE[key]


def _host_inputs(x, Wq, Wk, Wv, Wo):
    xT = np.ascontiguousarray(x[0].T).astype(np.float32)
    ii = np.arange(128)
    mneg = np.where(ii[None, :] > ii[:, None], np.float32(NEG_BIG),
                    np.float32(0.0)).astype(np.float32)
    m01 = (ii[None, :] <= ii[:, None]).astype(np.float32)
    m01u = (ii[None, :] >= ii[:, None]).astype(np.float16)
    in_maps = []
    for c in range(N_CORES):
        hsl = slice(128 * c, 128 * (c + 1))
        in_maps.append({
            "xT": xT,
            "wqT": np.ascontiguousarray((Wq[hsl] * np.float32(SCALE)).T).astype(np.float32),
            "wkT": np.ascontiguousarray(Wk[hsl].T).astype(np.float32),
            "wvT": np.ascontiguousarray(Wv[hsl].T).astype(np.float32),
            "woT": np.ascontiguousarray(Wo[:, hsl].T).astype(np.float32),
            "mneg": mneg,
            "m01": m01,
            "m01u": m01u,
            "ident": np.eye(128, dtype=np.float32),
        })
    return in_maps


def kernel(x, Wq, Wk, Wv, Wo, _trace=False, _debug=None):
    nc = _get_nc(_debug)
    in_maps = _host_inputs(np.asarray(x), np.asarray(Wq), np.asarray(Wk),
                           np.asarray(Wv), np.asarray(Wo))
    res = run_bass_kernel_spmd(nc, in_maps, core_ids=list(range(N_CORES)),
                               trace=_trace)
    out = np.zeros((L, D), np.float32)
    for c in range(N_CORES):
        out += res.results[c]["out"]
    if _trace:
        _CACHE["last_results"] = res
    return out.reshape(1, L, D)



# revision 25
# speedup vs baseline: 1.3315x; 1.3315x over previous
"""Entmax attention Trainium2 kernel v3 (8-core SPMD, head-parallel).

Math (matches reference _entmax_naive):
  scores = (q*scale) @ k^T  (fp16 hi/lo, 2-matmul stacked form, ~2^-21)
  per row: Gaussian-model warm start for tau root of sum relu(s-t) = 1
           (mu, sigma from rowsum / sum s^2; z via deg-5 poly of ln(n*sigma)),
           then 3x log-Newton (dlt = F*ln(F)/C) + 1 stale-slope step
           (Taylor ln near F=1), final count k = #{s > t}.
  tau_star = (rowsum - 1)/k;  rowsum computed exactly via prefix-scan of k
           vectors + per-block rowwise dot (PE), not an eviction pass.
  P~^T recomputed transposed via PE (khi+ones) @ (qhi + (-tau) row), relu.
  AV with V^T-stationary; Z from ones row; normalize at Wo eviction.

Scheduling: two super-groups split by row-block: B = rb 8..15 (emitted
first), A = rb 0..7. Iterations for both run lockstep (F on ACT, C on
DVE concurrently); ST/AV/Wo per half so the tail overlaps.

Sharding: 16 heads / 8 cores = 2 heads per core; host sums Wo partials
(fp16 partials, summed in fp32).
"""
import numpy as np
from contextlib import ExitStack

import concourse.bass as bass
import concourse.tile as tile
import concourse.mybir as mybir
from concourse import bacc
from concourse.bass_utils import run_bass_kernel_spmd

L = 2048
D = 1024
H = 16
HD = 64
N_CORES = 8
HPC = 2
SCALE = float(HD) ** -0.5

FP32 = mybir.dt.float32
FP16 = mybir.dt.float16
BF16 = mybir.dt.bfloat16
Alu = mybir.AluOpType
Act = mybir.ActivationFunctionType

NEG_BIG = -1.0e30
MAX_INIT = -3.0e38

B_PAIRS = [(8, 15), (9, 14), (10, 13), (11, 12)]   # 25 blocks each
A_PAIRS = [(0, 7), (1, 6), (2, 5), (3, 4)]         # 9 blocks each
# z(u) polynomial, u = ln(n*sigma), high->low
ZPOLY = [0.00066750066, -0.013271971, 0.10065626, -0.39364207,
         1.2942792, -0.90334845]
# PT bins: which jt's share a reused S_B tile (per head)
PT_BINS = [[0, 1, 2], [3, 4, 5, 6], [7, 8, 9, 10, 11, 12, 13, 14, 15]]


def col_of(rb, h):
    if rb >= 8:
        return 8 * h + (rb - 8)
    return 16 + 8 * h + rb


def units_of(pairs, h):
    """[(rb, off_in_tile, pair_index)]"""
    out = []
    for p, (ra, rb) in enumerate(pairs):
        out.append((ra, 0, p))
        out.append((rb, 128 * (ra + 1), p))
    return out


def build_program():
    nc = bacc.Bacc("TRN2", target_bir_lowering=False, debug=False, num_devices=1)

    xT_d = nc.dram_tensor("xT", [D, L], FP32, kind="ExternalInput")
    wqh_d = nc.dram_tensor("wqh", [D, 128], FP16, kind="ExternalInput")
    wql_d = nc.dram_tensor("wql", [D, 128], FP16, kind="ExternalInput")
    wkh_d = nc.dram_tensor("wkh", [D, 128], FP16, kind="ExternalInput")
    wkl_d = nc.dram_tensor("wkl", [D, 128], FP16, kind="ExternalInput")
    wvh_d = nc.dram_tensor("wvh", [D, 128], FP16, kind="ExternalInput")
    wo_d = nc.dram_tensor("woT", [128, D], FP16, kind="ExternalInput")
    mneg_d = nc.dram_tensor("mneg", [128, 128], FP32, kind="ExternalInput")
    m01u_d = nc.dram_tensor("m01u", [128, 128], FP16, kind="ExternalInput")
    ident_d = nc.dram_tensor("ident", [128, 128], FP32, kind="ExternalInput")
    # [128, 32] per-unit-column constants
    nvals_d = nc.dram_tensor("nvals", [128, 32], FP32, kind="ExternalInput")
    rn_d = nc.dram_tensor("rn", [128, 32], FP32, kind="ExternalInput")
    rn2_d = nc.dram_tensor("rn2", [128, 32], FP32, kind="ExternalInput")
    wvals_d = nc.dram_tensor("wvals", [128, 32], FP32, kind="ExternalInput")
    out_d = nc.dram_tensor("out", [L, D], FP16, kind="ExternalOutput")
    dbg_d = nc.dram_tensor("dbg", [128, 256], FP32, kind="ExternalOutput")

    with tile.TileContext(nc) as tc:
        with ExitStack() as ctx:
            persist = ctx.enter_context(tc.tile_pool(name="persist", bufs=1))
            # q/k fp16 tiles
            qstack = [persist.tile([128, L], FP16, tag=f"qst{h}", name=f"qst{h}")
                      for h in range(2)]
            qhx = [persist.tile([65, L], FP16, tag=f"qhx{h}", name=f"qhx{h}")
                   for h in range(2)]
            kdup = [persist.tile([128, L], FP16, tag=f"kd{h}", name=f"kd{h}")
                    for h in range(2)]
            ktl = [persist.tile([128, L], FP16, tag=f"ktl{h}", name=f"ktl{h}")
                   for h in range(2)]
            khx = [persist.tile([65, L], FP16, tag=f"khx{h}", name=f"khx{h}")
                   for h in range(2)]
            vt = persist.tile([128, 16, 130], FP16, tag="vt", name="vt")
            woTh = persist.tile([128, D], FP16, tag="woTh", name="woTh")
            zsb = persist.tile([65, L], FP32, tag="zsb", name="zsb")
            trash_ev = persist.tile([128, 1024], BF16, tag="trev", name="trev")
            mneg = persist.tile([128, 128], FP32, tag="mneg", name="mneg")
            m01u = persist.tile([128, 128], FP16, tag="m01u", name="m01u")
            ident = persist.tile([128, 128], FP32, tag="ident", name="ident")
            nvals = persist.tile([128, 32], FP32, tag="nvals", name="nvals")
            rn = persist.tile([128, 32], FP32, tag="rn", name="rn")
            rn2 = persist.tile([128, 32], FP32, tag="rn2", name="rn2")
            wvals = persist.tile([128, 32], FP32, tag="wvals", name="wvals")
            onesc = persist.tile([128, 1], FP32, tag="onesc", name="onesc")
            trash_d = persist.tile([128, 128], BF16, tag="trd", name="trd")
            ident16 = persist.tile([128, 128], FP16, tag="id16", name="id16")
            zeros_bf = persist.tile([128, L], BF16, tag="zbf", name="zbf")

            NST = 32

            def stat(tag):
                return persist.tile([128, NST], FP32, tag=tag, name=tag)

            maxF, maxG, maxD, mx = stat("maxF"), stat("maxG"), stat("maxD"), stat("mx")
            sqF, sqG, sumsq = stat("sqF"), stat("sqG"), stat("sumsq")
            rowsum = stat("rowsum")
            nT = stat("nT")
            Ft, Ct = stat("Ft"), stat("Ct")
            Fg, Cg = stat("Fg"), stat("Cg")
            lF, rec, dlt = stat("lF"), stat("rec"), stat("dlt")
            tm1, tm2, tm3 = stat("tm1"), stat("tm2"), stat("tm3")
            tau, ntau = stat("tau"), stat("ntau")
            Tt = stat("Tt")
            zps_s, rzt = stat("zps_s"), stat("rzt")
            dbg = persist.tile([128, 256], FP32, tag="dbg", name="dbg")

            def dbg_cp(sl0, src_t):
                nc.gpsimd.tensor_copy(dbg[:, sl0:sl0 + 32], src_t[:])

            nc.sync.dma_start(mneg[:], mneg_d.ap())
            nc.sync.dma_start(m01u[:], m01u_d.ap())
            nc.sync.dma_start(ident[:], ident_d.ap())
            nc.sync.dma_start(nvals[:], nvals_d.ap())
            nc.sync.dma_start(rn[:], rn_d.ap())
            nc.sync.dma_start(rn2[:], rn2_d.ap())
            nc.sync.dma_start(wvals[:], wvals_d.ap())
            nc.sync.dma_start(woTh[:], wo_d.ap())
            nc.vector.memset(maxF[:], MAX_INIT)
            nc.vector.memset(maxG[:], MAX_INIT)
            nc.vector.memset(maxD[:], MAX_INIT)
            nc.vector.memset(sqF[:], 0.0)
            nc.vector.memset(sqG[:], 0.0)
            nc.vector.memset(vt[:, :, 64:65], 1.0)
            nc.vector.memset(vt[:, :, 129:130], 1.0)
            nc.vector.memset(onesc[:], 1.0)
            nc.vector.memset(zeros_bf[:], 0.0)
            nc.scalar.copy(ident16[:], ident[:])
            for h in range(2):
                nc.vector.memset(khx[h][64:65, :], 1.0)
                nc.vector.memset(qhx[h][64:65, :], 0.0)

            copy_flip = [0]

            def balanced_copy(dst, src):
                if copy_flip[0] % 2 == 0:
                    nc.scalar.copy(dst, src)
                else:
                    nc.vector.tensor_copy(dst, src)
                copy_flip[0] += 1

            # ---------------- P1: projections (streamed x chunks) ----------
            with ExitStack() as p1:
                xp = p1.enter_context(tc.tile_pool(name="xp", bufs=2))
                wp = p1.enter_context(tc.tile_pool(name="wp", bufs=1))
                pfx = p1.enter_context(tc.tile_pool(name="pfx", bufs=1))
                p1ps = p1.enter_context(
                    tc.tile_pool(name="p1ps", bufs=2, space="PSUM"))
                p1ps2 = p1.enter_context(
                    tc.tile_pool(name="p1ps2", bufs=2, space="PSUM"))
                p1ps3 = p1.enter_context(
                    tc.tile_pool(name="p1ps3", bufs=1, space="PSUM"))

                wqh = wp.tile([128, 8, 128], FP16, tag="wqh", name="wqh")
                wql = wp.tile([128, 8, 128], FP16, tag="wql", name="wql")
                wkh = wp.tile([128, 8, 128], FP16, tag="wkh", name="wkh")
                wkl = wp.tile([128, 8, 128], FP16, tag="wkl", name="wkl")
                wvh = wp.tile([128, 8, 128], FP16, tag="wvh", name="wvh")
                vTs = wp.tile([128, L], FP16, tag="vTs", name="vTs")
                for wt_, wd_ in ((wqh, wqh_d), (wql, wql_d), (wkh, wkh_d),
                                 (wkl, wkl_d), (wvh, wvh_d)):
                    nc.sync.dma_start(
                        wt_[:], wd_.ap().rearrange("(c p) m -> p c m", p=128))

                q32 = pfx.tile([128, L], FP32, tag="q32", name="q32")
                k32 = pfx.tile([128, L], FP32, tag="k32", name="k32")
                kcum = pfx.tile([128, L], FP32, tag="kc", name="kc")
                rs_row = pfx.tile([65, L], FP32, tag="rsr", name="rsr")

                xview = xT_d.ap().rearrange("(c p) n -> p c n", p=128)
                for cchunk in range(4):
                    cs = slice(512 * cchunk, 512 * (cchunk + 1))
                    xt = xp.tile([128, 8, 512], FP32, tag="xt", name=f"xt{cchunk}")
                    xhi = xp.tile([128, 8, 512], FP16, tag="xhi", name=f"xhi{cchunk}")
                    xlo = xp.tile([128, 8, 512], FP16, tag="xlo", name=f"xlo{cchunk}")
                    nc.sync.dma_start(xt[:], xview[:, :, cs])
                    nc.scalar.copy(xhi[:], xt[:])
                    nc.vector.tensor_tensor(xlo[:], xt[:], xhi[:], Alu.subtract)

                    for which, whi, wlo in ((0, wqh, wql), (1, wkh, wkl)):
                        ps = p1ps.tile([128, 512], FP32, tag="pp",
                                       name=f"pj{which}_{cchunk}")
                        for e in range(8):
                            nc.tensor.matmul(ps[:], whi[:, e, :], xhi[:, e, :],
                                             start=(e == 0), stop=False)
                        for e in range(8):
                            nc.tensor.matmul(ps[:], whi[:, e, :], xlo[:, e, :],
                                             start=False, stop=False)
                        for e in range(8):
                            nc.tensor.matmul(ps[:], wlo[:, e, :], xhi[:, e, :],
                                             start=False, stop=(e == 7))
                        for h in range(2):
                            rs_ = slice(64 * h, 64 * h + 64)
                            if which == 0:
                                # q: hi -> qstack[0:64] & qhx, lo -> qstack[64:]
                                nc.scalar.copy(qstack[h][0:64, cs], ps[rs_, :])
                                nc.vector.tensor_tensor(
                                    qstack[h][64:128, cs], ps[rs_, :],
                                    qstack[h][0:64, cs], Alu.subtract)
                                nc.gpsimd.tensor_copy(qhx[h][0:64, cs],
                                                      qstack[h][0:64, cs])
                                nc.vector.tensor_copy(
                                    q32[64 * h:64 * h + 64, cs], ps[rs_, :])
                            else:
                                nc.scalar.copy(kdup[h][0:64, cs], ps[rs_, :])
                                nc.vector.tensor_tensor(
                                    ktl[h][0:64, cs], ps[rs_, :],
                                    kdup[h][0:64, cs], Alu.subtract)
                                nc.gpsimd.tensor_copy(kdup[h][64:128, cs],
                                                      kdup[h][0:64, cs])
                                nc.gpsimd.tensor_copy(khx[h][0:64, cs],
                                                      kdup[h][0:64, cs])
                                nc.scalar.copy(
                                    k32[64 * h:64 * h + 64, cs], ps[rs_, :])

                    # v chunk: vT orientation [d, l]
                    psv = p1ps2.tile([128, 512], FP32, tag="pv", name=f"pv{cchunk}")
                    for e in range(8):
                        nc.tensor.matmul(psv[:], wvh[:, e, :], xhi[:, e, :],
                                         start=(e == 0), stop=(e == 7))
                    balanced_copy(vTs[:, cs], psv[:])

                # v transposes: vT [d, j-block] -> vt[j, jt, d-cols]
                for jt in range(16):
                    jsl = slice(128 * jt, 128 * jt + 128)
                    pst = p1ps3.tile([128, 128], FP16, tag="pvt", name=f"pvt{jt}")
                    nc.tensor.transpose(pst[:], vTs[:, jsl], ident16[:])
                    balanced_copy(vt[:, jt, 0:64], pst[:, 0:64])
                    balanced_copy(vt[:, jt, 65:129], pst[:, 64:128])

                # q32 currently holds a copy of psum q (fp32) via bypass;
                # fix: q32 = qhi + qlo would lose nothing; q32 written above
                # directly from psum (bypass keeps in0). k32 likewise.
                # prefix scan of k32 -> kcum; rowsum = <q32, kcum> per block
                nc.vector.tensor_tensor_scan(
                    kcum[:], k32[:], k32[:], 0.0, Alu.add, Alu.bypass)
                nc.vector.tensor_tensor(q32[:], q32[:], kcum[:], Alu.mult)
                for h in range(2):
                    hp = slice(64 * h, 64 * h + 64)
                    for cc in range(4):
                        cs2 = slice(512 * cc, 512 * (cc + 1))
                        psr = p1ps3.tile([1, 512], FP32, tag="psr",
                                         name=f"psr{h}_{cc}")
                        nc.tensor.matmul(psr[:], onesc[hp], q32[hp, cs2],
                                         start=True, stop=True)
                        nc.scalar.copy(rs_row[64 * h:64 * h + 1, cs2], psr[:])
                # transpose rowsum row into stat cols
                psq = p1ps3.tile([128, 32], FP32, tag="psq", name="psq")
                for h in range(2):
                    for rb in range(16):
                        cl = col_of(rb, h)
                        nc.tensor.transpose(
                            psq[:, cl:cl + 1],
                            rs_row[64 * h:64 * h + 1, 128 * rb:128 * rb + 128],
                            ident[64 * h:64 * h + 1, 64 * h:64 * h + 1])
                nc.vector.tensor_copy(rowsum[:], psq[:])

            # ---------------- P2: scores + eviction ------------------------
            sB_pool = ctx.enter_context(tc.tile_pool(name="sB", bufs=1))
            SB = {}
            for h in range(2):
                for p in range(4):
                    SB[(p, h)] = sB_pool.tile([128, 3200], FP32,
                                              tag=f"sb{p}_{h}", name=f"sb{p}_{h}")

            def emit_scores(pairs, S_of, h, ps_pool):
                units = units_of(pairs, h)
                for (rb, off, p) in units:
                    col = col_of(rb, h)
                    n = 128 * (rb + 1)
                    S = S_of(p, h)
                    rbs = slice(128 * rb, 128 * rb + 128)
                    for ci, c0 in enumerate(range(0, n, 1024)):
                        w = min(1024, n - c0)
                        ps = ps_pool.tile([128, 1024], FP32, tag="sc",
                                          name=f"sc{rb}_{h}_{ci}")
                        for s0 in range(0, w, 512):
                            sw = min(512, w - s0)
                            cs = slice(c0 + s0, c0 + s0 + sw)
                            pss = ps[:, s0:s0 + sw]
                            nc.tensor.matmul(pss, qstack[h][:, rbs],
                                             kdup[h][:, cs],
                                             start=True, stop=False)
                            nc.tensor.matmul(pss, qstack[h][0:64, rbs],
                                             ktl[h][0:64, cs],
                                             start=False, stop=True)
                        last = (c0 + w == n)
                        mainw = w - 128 if last else w
                        m_acc = maxF if ci == 0 else maxG
                        s_acc = sqF if ci == 0 else sqG
                        # sum of squares over FULL chunk (incl diag garbage --
                        # only feeds the sigma estimate)
                        nc.scalar.activation(
                            trash_ev[:, :w], ps[:, :w], Act.Square,
                            bias=0.0, accum_out=s_acc[:, col:col + 1])
                        if mainw > 0:
                            nc.vector.tensor_scalar(
                                out=S[:, off + c0:off + c0 + mainw],
                                in0=ps[:, :mainw],
                                scalar1=0.0, scalar2=MAX_INIT,
                                op0=Alu.add, op1=Alu.max,
                                accum_out=m_acc[:, col:col + 1])
                        if last:
                            nc.vector.tensor_tensor(
                                S[:, off + n - 128:off + n],
                                ps[:, mainw:w], mneg[:], Alu.add)
                            nc.vector.tensor_scalar(
                                out=trash_d[:, :128],
                                in0=S[:, off + n - 128:off + n],
                                scalar1=0.0, scalar2=MAX_INIT,
                                op0=Alu.add, op1=Alu.max,
                                accum_out=maxD[:, col:col + 1])

            with ExitStack() as p2s:
                ps_sc = p2s.enter_context(
                    tc.tile_pool(name="ps_sc", bufs=3, space="PSUM"))
                for h in range(2):
                    emit_scores(B_PAIRS, lambda p, hh: SB[(p, hh)], h, ps_sc)

                sA_pool = ctx.enter_context(tc.tile_pool(name="sA", bufs=1))
                SA = {}
                for h in range(2):
                    for p in range(4):
                        SA[(p, h)] = sA_pool.tile([128, 1152], FP32,
                                                  tag=f"sa{p}_{h}",
                                                  name=f"sa{p}_{h}")
                for h in range(2):
                    emit_scores(A_PAIRS, lambda p, hh: SA[(p, hh)], h, ps_sc)

            # all units: (rb, off, tile)
            all_units = []
            for h in range(2):
                for (rb, off, p) in units_of(B_PAIRS, h):
                    all_units.append((rb, off, h, SB[(p, h)]))
                for (rb, off, p) in units_of(A_PAIRS, h):
                    all_units.append((rb, off, h, SA[(p, h)]))

            # engine split for half passes (balance ~136 blocks each):
            act_half = [(rb, off, h, S) for (rb, off, h, S) in all_units
                        if (rb >= 8) == (h == 0)]
            dve_half = [(rb, off, h, S) for (rb, off, h, S) in all_units
                        if (rb >= 8) != (h == 0)]

            # ---------------- model start ----------------------------------
            nc.vector.tensor_tensor(mx[:], maxF[:], maxG[:], Alu.max)
            nc.vector.tensor_tensor(mx[:], mx[:], maxD[:], Alu.max)
            nc.vector.tensor_tensor(sumsq[:], sqF[:], sqG[:], Alu.add)
            nc.gpsimd.tensor_tensor(tm1[:], rowsum[:], rn[:], Alu.mult)   # mu
            nc.gpsimd.tensor_tensor(tm2[:], sumsq[:], rn2[:], Alu.mult)   # E[s2]
            nc.gpsimd.tensor_tensor(tm3[:], tm1[:], tm1[:], Alu.mult)     # mu^2
            nc.gpsimd.tensor_tensor(tm2[:], tm2[:], tm3[:], Alu.subtract)  # var
            nc.vector.tensor_scalar_max(tm2[:], tm2[:], 1.0e-12)
            nc.scalar.activation(tm3[:], tm2[:], Act.Sqrt, bias=0.0)      # sigma
            nc.gpsimd.tensor_tensor(tm2[:], tm3[:], nvals[:], Alu.mult)   # w
            nc.vector.tensor_scalar_max(tm2[:], tm2[:], 0.3)
            nc.scalar.activation(Fg[:], tm2[:], Act.Ln, bias=0.0)         # u
            # Horner z = poly(u) -> dlt
            nc.gpsimd.tensor_scalar(out=dlt[:], in0=Fg[:], scalar1=ZPOLY[0],
                                    scalar2=ZPOLY[1], op0=Alu.mult, op1=Alu.add)
            for c in ZPOLY[2:]:
                nc.gpsimd.tensor_tensor(dlt[:], dlt[:], Fg[:], Alu.mult)
                nc.gpsimd.tensor_scalar_add(dlt[:], dlt[:], float(c))
            nc.gpsimd.tensor_tensor(dlt[:], dlt[:], tm3[:], Alu.mult)     # sg*z
            nc.gpsimd.tensor_tensor(dlt[:], dlt[:], tm1[:], Alu.add)      # t0
            nc.gpsimd.tensor_scalar_add(tm1[:], mx[:], -1.0)
            nc.vector.tensor_tensor(dlt[:], dlt[:], tm1[:], Alu.max)
            nc.gpsimd.tensor_scalar_add(tm1[:], mx[:], -0.001)
            nc.vector.tensor_tensor(Tt[:], dlt[:], tm1[:], Alu.min)
            nc.gpsimd.tensor_scalar_mul(nT[:], Tt[:], -1.0)
            dbg_cp(0, rowsum)
            dbg_cp(32, sumsq)
            dbg_cp(64, Tt)
            dbg_cp(96, mx)

            # ---------------- iterations -----------------------------------
            def f_pass_act(units):
                for (rb, off, h, S) in units:
                    col = col_of(rb, h)
                    n = 128 * (rb + 1)
                    nc.scalar.activation(
                        trash_a[:, :n], S[:, off:off + n], Act.Relu,
                        bias=nT[:, col:col + 1],
                        accum_out=Ft[:, col:col + 1])

            def f_pass_dve(units):
                for (rb, off, h, S) in units:
                    col = col_of(rb, h)
                    n = 128 * (rb + 1)
                    nc.vector.scalar_tensor_tensor(
                        out=trash_c[:, :n], in0=S[:, off:off + n],
                        scalar=nT[:, col:col + 1], in1=zeros_bf[:, :n],
                        op0=Alu.add, op1=Alu.max,
                        accum_out=Ft[:, col:col + 1])

            def c_pass_dve(units):
                for (rb, off, h, S) in units:
                    col = col_of(rb, h)
                    n = 128 * (rb + 1)
                    nc.vector.tensor_scalar(
                        out=trash_c[:, :n], in0=S[:, off:off + n],
                        scalar1=Tt[:, col:col + 1], scalar2=0.0,
                        op0=Alu.is_gt, op1=Alu.add,
                        accum_out=Ct[:, col:col + 1])

            def c_pass_act(units):
                for (rb, off, h, S) in units:
                    col = col_of(rb, h)
                    n = 128 * (rb + 1)
                    nc.scalar.activation(
                        trash_a[:, :n], S[:, off:off + n], Act.Sign,
                        bias=nT[:, col:col + 1],
                        accum_out=Ct[:, col:col + 1])

            # trash tiles reuse score-phase fp16 tiles (same byte footprint)
            trash_a = persist.tile([128, L], BF16, tag="ktl0", name="trash_a")
            trash_c = persist.tile([128, L], BF16, tag="kd0", name="trash_c")

            def taylor_ln():
                # lF = (Fg-1)*(1 - 0.5*(Fg-1))
                nc.gpsimd.tensor_scalar_add(tm1[:], Fg[:], -1.0)
                nc.gpsimd.tensor_scalar(out=tm2[:], in0=tm1[:], scalar1=-0.5,
                                        scalar2=1.0, op0=Alu.mult, op1=Alu.add)
                nc.gpsimd.tensor_tensor(lF[:], tm1[:], tm2[:], Alu.mult)

            for it in range(3):
                f_pass_act(all_units)
                c_pass_dve(all_units)
                nc.vector.tensor_scalar_max(Fg[:], Ft[:], 1.0e-12)
                if it < 2:
                    nc.scalar.activation(lF[:], Fg[:], Act.Ln, bias=0.0)
                else:
                    taylor_ln()
                nc.vector.tensor_scalar_max(Cg[:], Ct[:], 1.0)
                nc.vector.reciprocal(rec[:], Cg[:])
                nc.gpsimd.tensor_tensor(dlt[:], lF[:], Fg[:], Alu.mult)
                nc.gpsimd.tensor_tensor(dlt[:], dlt[:], rec[:], Alu.mult)
                nc.vector.tensor_scalar(out=dlt[:], in0=dlt[:], scalar1=-1.0,
                                        scalar2=1.0, op0=Alu.max, op1=Alu.min)
                nc.gpsimd.tensor_tensor(nT[:], nT[:], dlt[:], Alu.subtract)
                nc.gpsimd.tensor_tensor(Tt[:], Tt[:], dlt[:], Alu.add)

            # stale-slope step (F only, split engines; rec from iter 3)
            f_pass_act(act_half)
            f_pass_dve(dve_half)
            nc.vector.tensor_scalar_max(Fg[:], Ft[:], 1.0e-12)
            taylor_ln()
            nc.gpsimd.tensor_tensor(dlt[:], lF[:], Fg[:], Alu.mult)
            nc.gpsimd.tensor_tensor(dlt[:], dlt[:], rec[:], Alu.mult)
            nc.vector.tensor_scalar(out=dlt[:], in0=dlt[:], scalar1=-1.0,
                                    scalar2=1.0, op0=Alu.max, op1=Alu.min)
            nc.gpsimd.tensor_tensor(nT[:], nT[:], dlt[:], Alu.subtract)
            nc.gpsimd.tensor_tensor(Tt[:], Tt[:], dlt[:], Alu.add)

            # final count (split engines; ACT via Sign trick)
            c_pass_act(act_half)
            c_pass_dve(dve_half)
            # fix ACT cols: C = 0.5*C + wvals  (wvals = 64*(rb+1))
            for sl in (slice(0, 8), slice(24, 32)):
                nc.vector.scalar_tensor_tensor(
                    out=Ct[:, sl], in0=Ct[:, sl], scalar=0.5, in1=wvals[:, sl],
                    op0=Alu.mult, op1=Alu.add)
            nc.vector.tensor_scalar_max(Cg[:], Ct[:], 1.0)
            nc.vector.reciprocal(rec[:], Cg[:])
            nc.gpsimd.tensor_scalar_add(tm1[:], rowsum[:], -1.0)
            nc.gpsimd.tensor_tensor(tau[:], tm1[:], rec[:], Alu.mult)
            nc.gpsimd.tensor_scalar_mul(ntau[:], tau[:], -1.0)
            dbg_cp(128, Tt)
            dbg_cp(160, Cg)
            dbg_cp(192, tau)

            # ---------------- route -tau into qhx row 64 -------------------
            with ExitStack() as pr:
                ps_r = pr.enter_context(
                    tc.tile_pool(name="ps_r", bufs=2, space="PSUM"))
                st_r = pr.enter_context(tc.tile_pool(name="st_r", bufs=2))
                for sg, base in (("B", 0), ("A", 16)):
                    for h in range(2):
                        sl8 = slice(base + 8 * h, base + 8 * h + 8)
                        xf = st_r.tile([128, 8], FP32, tag=f"xf{h}",
                                       name=f"xf{sg}_{h}")
                        nc.vector.tensor_copy(xf[:], ntau[:, sl8])
                        psx = ps_r.tile([8, 128], FP32, tag="psx",
                                        name=f"psx{sg}_{h}")
                        nc.tensor.transpose(psx[:], xf[:], ident[:])
                        stg = st_r.tile([8, 128], FP32, tag=f"stg{h}",
                                        name=f"stg{sg}_{h}")
                        nc.scalar.copy(stg[:], psx[:])
                        p64 = ps_r.tile([1, 1024], FP32, tag="p64",
                                        name=f"p64{sg}_{h}")
                        for b in range(8):
                            nc.tensor.matmul(
                                p64[:, 128 * b:128 * (b + 1)],
                                ident[0:8, b:b + 1], stg[:],
                                start=True, stop=True)
                        if sg == "B":
                            osl = slice(1024, 2048)
                        else:
                            osl = slice(0, 1024)
                        nc.scalar.copy(qhx[h][64:65, osl], p64[:])

            # ---------------- ST + AV + Wo ---------------------------------
            # PT bins reuse SB tiles (pads to same bytes: 6400 fp16 cols)
            pt_off = {}
            PT = {}
            for b, jts in enumerate(PT_BINS):
                off = 0
                for jt in jts:
                    pt_off[jt] = (b, off)
                    off += (16 - jt) * 128
            for h in range(2):
                for b in range(3):
                    PT[(h, b)] = sB_pool.tile([128, 6400], FP16,
                                              tag=f"sb{b}_{h}",
                                              name=f"pt{h}_{b}")

            outTh = persist.tile([128, L], FP16, tag="qst0", name="outTh")

            with ExitStack() as p3:
                ps_st = p3.enter_context(
                    tc.tile_pool(name="ps_st", bufs=2, space="PSUM"))
                ps_av = p3.enter_context(
                    tc.tile_pool(name="ps_av", bufs=1, space="PSUM"))
                ps_wo = p3.enter_context(
                    tc.tile_pool(name="ps_wo", bufs=1, space="PSUM"))
                ps_z = p3.enter_context(
                    tc.tile_pool(name="ps_z", bufs=1, space="PSUM"))
                wo_pool = p3.enter_context(tc.tile_pool(name="wop", bufs=2))

                def emit_st(h, jt, i_lo, i_hi):
                    """P~^T[j in jt-block, i in [i_lo, i_hi)] into PT bin."""
                    b, off = pt_off[jt]
                    pt_tile = PT[(h, b)]
                    jsl = slice(128 * jt, 128 * jt + 128)
                    for c0 in range(i_lo, i_hi, 512):
                        cw = min(512, i_hi - c0)
                        ps = ps_st.tile([128, 512], FP32, tag="st",
                                        name=f"st{h}_{jt}_{c0}")
                        isl = slice(c0, c0 + cw)
                        nc.tensor.matmul(ps[:, :cw], khx[h][:, jsl],
                                         qhx[h][:, isl], start=True, stop=True)
                        d0 = 128 if c0 == 128 * jt else 0
                        po = off + (c0 - 128 * jt)
                        if d0:
                            nc.vector.scalar_tensor_tensor(
                                out=pt_tile[:, po:po + 128],
                                in0=ps[:, 0:128], scalar=0.0, in1=m01u[:],
                                op0=Alu.max, op1=Alu.mult)
                        if cw > d0:
                            if (jt + h) % 2 == 0:
                                nc.scalar.activation(
                                    pt_tile[:, po + d0:po + cw],
                                    ps[:, d0:cw], Act.Relu, bias=0.0)
                            else:
                                nc.vector.tensor_scalar(
                                    out=pt_tile[:, po + d0:po + cw],
                                    in0=ps[:, d0:cw], scalar1=0.0,
                                    scalar2=0.0, op0=Alu.max, op1=Alu.add)

                def emit_av(h, half):
                    """out^T[d, i] for i in half (0: <1024, 1: >=1024)."""
                    i_lo_h = 1024 * half
                    i_hi_h = 1024 * (half + 1)
                    avp = ps_av.tile([65, 1024], FP32, tag="av",
                                     name=f"av{h}_{half}")
                    jts = [jt for jt in range(16) if 128 * jt < i_hi_h]
                    first = True
                    for jt in jts:
                        b, off = pt_off[jt]
                        lo = max(i_lo_h, 128 * jt)
                        for cc in range(lo, i_hi_h, 512):
                            ce = min(cc + 512, i_hi_h)
                            src = PT[(h, b)][:, off + (cc - 128 * jt):
                                             off + (ce - 128 * jt)]
                            nc.tensor.matmul(
                                avp[:, cc - i_lo_h:ce - i_lo_h],
                                vt[:, jt, 65 * h:65 * h + 65], src,
                                start=first,
                                stop=(jt == jts[-1] and ce == i_hi_h))
                        first = False
                    balanced_copy(outTh[64 * h:64 * h + 64, i_lo_h:i_hi_h],
                                  avp[0:64, :])
                    nc.scalar.copy(zsb[64 * h:64 * h + 1, i_lo_h:i_hi_h],
                                   avp[64:65, :])

                def emit_wo(blk):
                    wo_sb = wo_pool.tile([128, D], FP16, tag="wod",
                                         name=f"wod{blk}")
                    for oc in range(2):
                        osl2 = slice(512 * oc, 512 * (oc + 1))
                        wop0 = ps_wo.tile([128, 512], FP32, tag="wo0",
                                          name=f"wo0_{blk}_{oc}")
                        wop1 = ps_wo.tile([128, 512], FP32, tag="wo1",
                                          name=f"wo1_{blk}_{oc}")
                        nc.tensor.matmul(
                            wop0[:], outTh[0:64, 128 * blk:128 * (blk + 1)],
                            woTh[0:64, osl2], start=True, stop=True)
                        nc.tensor.matmul(
                            wop1[:], outTh[64:128, 128 * blk:128 * (blk + 1)],
                            woTh[64:128, osl2], start=True, stop=True)
                        nc.scalar.activation(
                            wo_sb[:, osl2], wop0[:], Act.Copy,
                            bias=0.0, scale=rzt[:, blk:blk + 1])
                        nc.vector.scalar_tensor_tensor(
                            out=wo_sb[:, osl2], in0=wop1[:],
                            scalar=rzt[:, 16 + blk:16 + blk + 1],
                            in1=wo_sb[:, osl2], op0=Alu.mult, op1=Alu.add)
                    nc.sync.dma_start(
                        out_d.ap()[128 * blk:128 * (blk + 1), :], wo_sb[:])

                # ---- half 1 (i >= 1024): ST_B -> AVh1 -> Z -> wo 8..15
                for h in range(2):
                    for jt in range(16):
                        lo = max(1024, 128 * jt)
                        emit_st(h, jt, lo, 2048)
                for h in range(2):
                    emit_av(h, 1)
                zp1 = ps_z.tile([128, 16], FP32, tag="zp", name="zp1")
                for h in range(2):
                    for b in range(8, 16):
                        nc.tensor.transpose(
                            zp1[:, 8 * h + b - 8:8 * h + b - 7],
                            zsb[64 * h:64 * h + 1, 128 * b:128 * b + 128],
                            ident[64 * h:64 * h + 1, 64 * h:64 * h + 1])
                for h in range(2):
                    dsl = slice(16 * h + 8, 16 * h + 16)
                    ssl = slice(8 * h, 8 * h + 8)
                    nc.vector.tensor_scalar_add(zps_s[:, dsl], zp1[:, ssl],
                                                1.0e-10)
                    nc.vector.reciprocal(rzt[:, dsl], zps_s[:, dsl])
                for blk in range(8, 16):
                    emit_wo(blk)

                # ---- half 0 (i < 1024): ST_A -> AVh0 -> Z -> wo 0..7
                for h in range(2):
                    for jt in range(8):
                        emit_st(h, jt, 128 * jt, 1024)
                for h in range(2):
                    emit_av(h, 0)
                zp0 = ps_z.tile([128, 16], FP32, tag="zp", name="zp0")
                for h in range(2):
                    for b in range(0, 8):
                        nc.tensor.transpose(
                            zp0[:, 8 * h + b:8 * h + b + 1],
                            zsb[64 * h:64 * h + 1, 128 * b:128 * b + 128],
                            ident[64 * h:64 * h + 1, 64 * h:64 * h + 1])
                for h in range(2):
                    dsl = slice(16 * h, 16 * h + 8)
                    ssl = slice(8 * h, 8 * h + 8)
                    nc.vector.tensor_scalar_add(zps_s[:, dsl], zp0[:, ssl],
                                                1.0e-10)
                    nc.vector.reciprocal(rzt[:, dsl], zps_s[:, dsl])
                for blk in range(0, 8):
                    emit_wo(blk)
                dbg_cp(224, rzt)
                nc.sync.dma_start(dbg_d.ap(), dbg[:])

    nc.compile()
    return nc


_CACHE = {}


def _get_nc():
    if "nc" not in _CACHE:
        _CACHE["nc"] = build_program()
    return _CACHE["nc"]


def _split16(a):
    hi = a.astype(np.float16)
    lo = (a.astype(np.float32) - hi.astype(np.float32)).astype(np.float16)
    return hi, lo


def _host_inputs(x, Wq, Wk, Wv, Wo):
    xT = np.ascontiguousarray(x[0].T).astype(np.float32)
    ii = np.arange(128)
    mneg = np.where(ii[None, :] > ii[:, None], np.float32(NEG_BIG),
                    np.float32(0.0)).astype(np.float32)
    m01u = (ii[None, :] >= ii[:, None]).astype(np.float16)
    ident = np.eye(128, dtype=np.float32)
    # per-unit-column constants [128, 32]
    nvals = np.zeros((128, 32), np.float32)
    rn = np.zeros((128, 32), np.float32)
    rn2 = np.zeros((128, 32), np.float32)
    wvals = np.zeros((128, 32), np.float32)
    r = np.arange(128, dtype=np.float32)
    for h in range(2):
        for rb in range(16):
            cl = col_of(rb, h)
            n = 128 * rb + r + 1.0
            nvals[:, cl] = n
            rn[:, cl] = 1.0 / n
            rn2[:, cl] = 1.0 / (128.0 * (rb + 1))
            wvals[:, cl] = 64.0 * (rb + 1)
    in_maps = []
    for c in range(N_CORES):
        hsl = slice(128 * c, 128 * (c + 1))
        wqh, wql = _split16((Wq[hsl] * np.float32(SCALE)).T)
        wkh, wkl = _split16(Wk[hsl].T)
        wvh, _ = _split16(Wv[hsl].T)
        in_maps.append({
            "xT": xT,
            "wqh": np.ascontiguousarray(wqh),
            "wql": np.ascontiguousarray(wql),
            "wkh": np.ascontiguousarray(wkh),
            "wkl": np.ascontiguousarray(wkl),
            "wvh": np.ascontiguousarray(wvh),
            "woT": np.ascontiguousarray(Wo[:, hsl].T).astype(np.float16),
            "mneg": mneg,
            "m01u": m01u,
            "ident": ident,
            "nvals": nvals,
            "rn": rn,
            "rn2": rn2,
            "wvals": wvals,
        })
    return in_maps


def kernel(x, Wq, Wk, Wv, Wo, _trace=False, _debug=None):
    nc = _get_nc()
    in_maps = _host_inputs(np.asarray(x), np.asarray(Wq), np.asarray(Wk),
                           np.asarray(Wv), np.asarray(Wo))
    res = run_bass_kernel_spmd(nc, in_maps, core_ids=list(range(N_CORES)),
                               trace=_trace)
    out = np.zeros((L, D), np.float32)
    for c in range(N_CORES):
        out += res.results[c]["out"].astype(np.float32)
    if _trace:
        _CACHE["last_results"] = res
    return out.reshape(1, L, D)


# revision 26
# speedup vs baseline: 1.3391x; 1.0057x over previous
"""Entmax attention Trainium2 kernel v3 (8-core SPMD, head-parallel).

Math (matches reference _entmax_naive):
  scores = (q*scale) @ k^T  (fp16 hi/lo, 2-matmul stacked form, ~2^-21)
  per row: Gaussian-model warm start for tau root of sum relu(s-t) = 1
           (mu, sigma from rowsum / sum s^2; z via deg-5 poly of ln(n*sigma)),
           then 3x log-Newton (dlt = F*ln(F)/C) + 1 stale-slope step
           (Taylor ln near F=1), final count k = #{s > t}.
  tau_star = (rowsum - 1)/k;  rowsum computed exactly via prefix-scan of k
           vectors + per-block rowwise dot (PE), not an eviction pass.
  P~^T recomputed transposed via PE (khi+ones) @ (qhi + (-tau) row), relu.
  AV with V^T-stationary; Z from ones row; normalize at Wo eviction.

Scheduling: two super-groups split by row-block: B = rb 8..15 (emitted
first), A = rb 0..7. Iterations for both run lockstep (F on ACT, C on
DVE concurrently); ST/AV/Wo per half so the tail overlaps.

Sharding: 16 heads / 8 cores = 2 heads per core; host sums Wo partials
(fp16 partials, summed in fp32).
"""
import numpy as np
from contextlib import ExitStack

import concourse.bass as bass
import concourse.tile as tile
import concourse.mybir as mybir
from concourse import bacc
from concourse.bass_utils import run_bass_kernel_spmd

L = 2048
D = 1024
H = 16
HD = 64
N_CORES = 8
HPC = 2
SCALE = float(HD) ** -0.5

FP32 = mybir.dt.float32
FP16 = mybir.dt.float16
BF16 = mybir.dt.bfloat16
Alu = mybir.AluOpType
Act = mybir.ActivationFunctionType

NEG_BIG = -1.0e30
MAX_INIT = -3.0e38

B_PAIRS = [(8, 15), (9, 14), (10, 13), (11, 12)]   # 25 blocks each
A_PAIRS = [(0, 7), (1, 6), (2, 5), (3, 4)]         # 9 blocks each
# z(u) polynomial, u = ln(n*sigma), high->low
ZPOLY = [0.00066750066, -0.013271971, 0.10065626, -0.39364207,
         1.2942792, -0.90334845]
# PT bins: which jt's share a reused S_B tile (per head)
PT_BINS = [[0, 1, 2], [3, 4, 5, 6], [7, 8, 9, 10, 11, 12, 13, 14, 15]]


def col_of(rb, h):
    if rb >= 8:
        return 8 * h + (rb - 8)
    return 16 + 8 * h + rb


def units_of(pairs, h):
    """[(rb, off_in_tile, pair_index)]"""
    out = []
    for p, (ra, rb) in enumerate(pairs):
        out.append((ra, 0, p))
        out.append((rb, 128 * (ra + 1), p))
    return out


def build_program():
    nc = bacc.Bacc("TRN2", target_bir_lowering=False, debug=False, num_devices=1)

    xT_d = nc.dram_tensor("xT", [D, L], FP32, kind="ExternalInput")
    wqh_d = nc.dram_tensor("wqh", [D, 128], FP16, kind="ExternalInput")
    wql_d = nc.dram_tensor("wql", [D, 128], FP16, kind="ExternalInput")
    wkh_d = nc.dram_tensor("wkh", [D, 128], FP16, kind="ExternalInput")
    wkl_d = nc.dram_tensor("wkl", [D, 128], FP16, kind="ExternalInput")
    wvh_d = nc.dram_tensor("wvh", [D, 128], FP16, kind="ExternalInput")
    wo_d = nc.dram_tensor("woT", [128, D], FP16, kind="ExternalInput")
    mneg_d = nc.dram_tensor("mneg", [128, 128], FP32, kind="ExternalInput")
    m01u_d = nc.dram_tensor("m01u", [128, 128], FP16, kind="ExternalInput")
    ident_d = nc.dram_tensor("ident", [128, 128], FP32, kind="ExternalInput")
    # [128, 32] per-unit-column constants
    nvals_d = nc.dram_tensor("nvals", [128, 32], FP32, kind="ExternalInput")
    rn_d = nc.dram_tensor("rn", [128, 32], FP32, kind="ExternalInput")
    rn2_d = nc.dram_tensor("rn2", [128, 32], FP32, kind="ExternalInput")
    wvals_d = nc.dram_tensor("wvals", [128, 32], FP32, kind="ExternalInput")
    out_d = nc.dram_tensor("out", [L, D], FP16, kind="ExternalOutput")

    with tile.TileContext(nc) as tc:
        with ExitStack() as ctx:
            persist = ctx.enter_context(tc.tile_pool(name="persist", bufs=1))
            # q/k fp16 tiles
            qstack = [persist.tile([128, L], FP16, tag=f"qst{h}", name=f"qst{h}")
                      for h in range(2)]
            qhx = [persist.tile([65, L], FP16, tag=f"qhx{h}", name=f"qhx{h}")
                   for h in range(2)]
            kdup = [persist.tile([128, L], FP16, tag=f"kd{h}", name=f"kd{h}")
                    for h in range(2)]
            ktl = [persist.tile([128, L], FP16, tag=f"ktl{h}", name=f"ktl{h}")
                   for h in range(2)]
            khx = [persist.tile([65, L], FP16, tag=f"khx{h}", name=f"khx{h}")
                   for h in range(2)]
            vt = persist.tile([128, 16, 130], FP16, tag="vt", name="vt")
            woTh = persist.tile([128, D], FP16, tag="woTh", name="woTh")
            zsb = persist.tile([65, L], FP32, tag="zsb", name="zsb")
            trash_ev = persist.tile([128, 1024], BF16, tag="trev", name="trev")
            mneg = persist.tile([128, 128], FP32, tag="mneg", name="mneg")
            m01u = persist.tile([128, 128], FP16, tag="m01u", name="m01u")
            ident = persist.tile([128, 128], FP32, tag="ident", name="ident")
            nvals = persist.tile([128, 32], FP32, tag="nvals", name="nvals")
            rn = persist.tile([128, 32], FP32, tag="rn", name="rn")
            rn2 = persist.tile([128, 32], FP32, tag="rn2", name="rn2")
            wvals = persist.tile([128, 32], FP32, tag="wvals", name="wvals")
            onesc = persist.tile([128, 1], FP32, tag="onesc", name="onesc")
            trash_d = persist.tile([128, 128], BF16, tag="trd", name="trd")
            ident16 = persist.tile([128, 128], FP16, tag="id16", name="id16")
            zeros_bf = persist.tile([128, L], BF16, tag="zbf", name="zbf")

            NST = 32

            def stat(tag):
                return persist.tile([128, NST], FP32, tag=tag, name=tag)

            maxF, maxG, maxD, mx = stat("maxF"), stat("maxG"), stat("maxD"), stat("mx")
            sqF, sqG, sumsq = stat("sqF"), stat("sqG"), stat("sumsq")
            rowsum = stat("rowsum")
            nT = stat("nT")
            Ft, Ct = stat("Ft"), stat("Ct")
            Fg, Cg = stat("Fg"), stat("Cg")
            lF, rec, dlt = stat("lF"), stat("rec"), stat("dlt")
            tm1, tm2, tm3 = stat("tm1"), stat("tm2"), stat("tm3")
            tau, ntau = stat("tau"), stat("ntau")
            Tt = stat("Tt")
            zps_s, rzt = stat("zps_s"), stat("rzt")

            def dbg_cp(sl0, src_t):
                pass

            nc.sync.dma_start(mneg[:], mneg_d.ap())
            nc.sync.dma_start(m01u[:], m01u_d.ap())
            nc.sync.dma_start(ident[:], ident_d.ap())
            nc.sync.dma_start(nvals[:], nvals_d.ap())
            nc.sync.dma_start(rn[:], rn_d.ap())
            nc.sync.dma_start(rn2[:], rn2_d.ap())
            nc.sync.dma_start(wvals[:], wvals_d.ap())
            nc.sync.dma_start(woTh[:], wo_d.ap())
            nc.vector.memset(maxF[:], MAX_INIT)
            nc.vector.memset(maxG[:], MAX_INIT)
            nc.vector.memset(maxD[:], MAX_INIT)
            nc.vector.memset(sqF[:], 0.0)
            nc.vector.memset(sqG[:], 0.0)
            nc.vector.memset(vt[:, :, 64:65], 1.0)
            nc.vector.memset(vt[:, :, 129:130], 1.0)
            nc.vector.memset(onesc[:], 1.0)
            nc.vector.memset(zeros_bf[:], 0.0)
            nc.scalar.copy(ident16[:], ident[:])
            for h in range(2):
                nc.vector.memset(khx[h][64:65, :], 1.0)
                nc.vector.memset(qhx[h][64:65, :], 0.0)

            copy_flip = [0]

            def balanced_copy(dst, src):
                if copy_flip[0] % 2 == 0:
                    nc.scalar.copy(dst, src)
                else:
                    nc.vector.tensor_copy(dst, src)
                copy_flip[0] += 1

            # ---------------- P1: projections (streamed x chunks) ----------
            with ExitStack() as p1:
                xp = p1.enter_context(tc.tile_pool(name="xp", bufs=2))
                wp = p1.enter_context(tc.tile_pool(name="wp", bufs=1))
                pfx = p1.enter_context(tc.tile_pool(name="pfx", bufs=1))
                p1ps = p1.enter_context(
                    tc.tile_pool(name="p1ps", bufs=2, space="PSUM"))
                p1ps2 = p1.enter_context(
                    tc.tile_pool(name="p1ps2", bufs=2, space="PSUM"))
                p1ps3 = p1.enter_context(
                    tc.tile_pool(name="p1ps3", bufs=1, space="PSUM"))

                wqh = wp.tile([128, 8, 128], FP16, tag="wqh", name="wqh")
                wql = wp.tile([128, 8, 128], FP16, tag="wql", name="wql")
                wkh = wp.tile([128, 8, 128], FP16, tag="wkh", name="wkh")
                wkl = wp.tile([128, 8, 128], FP16, tag="wkl", name="wkl")
                wvh = wp.tile([128, 8, 128], FP16, tag="wvh", name="wvh")
                vTs = wp.tile([128, L], FP16, tag="vTs", name="vTs")
                for wt_, wd_ in ((wqh, wqh_d), (wql, wql_d), (wkh, wkh_d),
                                 (wkl, wkl_d), (wvh, wvh_d)):
                    nc.sync.dma_start(
                        wt_[:], wd_.ap().rearrange("(c p) m -> p c m", p=128))

                q32 = pfx.tile([128, L], FP32, tag="q32", name="q32")
                k32 = pfx.tile([128, L], FP32, tag="k32", name="k32")
                kcum = pfx.tile([128, L], FP32, tag="kc", name="kc")
                rs_row = pfx.tile([65, L], FP32, tag="rsr", name="rsr")

                xview = xT_d.ap().rearrange("(c p) n -> p c n", p=128)
                for cchunk in range(4):
                    cs = slice(512 * cchunk, 512 * (cchunk + 1))
                    xt = xp.tile([128, 8, 512], FP32, tag="xt", name=f"xt{cchunk}")
                    xhi = xp.tile([128, 8, 512], FP16, tag="xhi", name=f"xhi{cchunk}")
                    xlo = xp.tile([128, 8, 512], FP16, tag="xlo", name=f"xlo{cchunk}")
                    nc.sync.dma_start(xt[:], xview[:, :, cs])
                    nc.scalar.copy(xhi[:], xt[:])
                    nc.vector.tensor_tensor(xlo[:], xt[:], xhi[:], Alu.subtract)

                    for which, whi, wlo in ((0, wqh, wql), (1, wkh, wkl)):
                        ps = p1ps.tile([128, 512], FP32, tag="pp",
                                       name=f"pj{which}_{cchunk}")
                        for e in range(8):
                            nc.tensor.matmul(ps[:], whi[:, e, :], xhi[:, e, :],
                                             start=(e == 0), stop=False)
                        for e in range(8):
                            nc.tensor.matmul(ps[:], whi[:, e, :], xlo[:, e, :],
                                             start=False, stop=False)
                        for e in range(8):
                            nc.tensor.matmul(ps[:], wlo[:, e, :], xhi[:, e, :],
                                             start=False, stop=(e == 7))
                        for h in range(2):
                            rs_ = slice(64 * h, 64 * h + 64)
                            if which == 0:
                                # q: hi -> qstack[0:64] & qhx, lo -> qstack[64:]
                                nc.scalar.copy(qstack[h][0:64, cs], ps[rs_, :])
                                nc.vector.tensor_tensor(
                                    qstack[h][64:128, cs], ps[rs_, :],
                                    qstack[h][0:64, cs], Alu.subtract)
                                nc.scalar.copy(qhx[h][0:64, cs],
                                               qstack[h][0:64, cs])
                                nc.vector.tensor_copy(
                                    q32[64 * h:64 * h + 64, cs], ps[rs_, :])
                            else:
                                nc.scalar.copy(kdup[h][0:64, cs], ps[rs_, :])
                                nc.vector.tensor_tensor(
                                    ktl[h][0:64, cs], ps[rs_, :],
                                    kdup[h][0:64, cs], Alu.subtract)
                                nc.vector.tensor_copy(kdup[h][64:128, cs],
                                                       kdup[h][0:64, cs])
                                nc.scalar.copy(khx[h][0:64, cs],
                                               kdup[h][0:64, cs])
                                nc.scalar.copy(
                                    k32[64 * h:64 * h + 64, cs], ps[rs_, :])

                    # v chunk: vT orientation [d, l]
                    psv = p1ps2.tile([128, 512], FP32, tag="pv", name=f"pv{cchunk}")
                    for e in range(8):
                        nc.tensor.matmul(psv[:], wvh[:, e, :], xhi[:, e, :],
                                         start=(e == 0), stop=(e == 7))
                    balanced_copy(vTs[:, cs], psv[:])

                # v transposes: vT [d, j-block] -> vt[j, jt, d-cols]
                for jt in range(16):
                    jsl = slice(128 * jt, 128 * jt + 128)
                    pst = p1ps3.tile([128, 128], FP16, tag="pvt", name=f"pvt{jt}")
                    nc.tensor.transpose(pst[:], vTs[:, jsl], ident16[:])
                    balanced_copy(vt[:, jt, 0:64], pst[:, 0:64])
                    balanced_copy(vt[:, jt, 65:129], pst[:, 64:128])

                # q32 currently holds a copy of psum q (fp32) via bypass;
                # fix: q32 = qhi + qlo would lose nothing; q32 written above
                # directly from psum (bypass keeps in0). k32 likewise.
                # prefix scan of k32 -> kcum; rowsum = <q32, kcum> per block
                nc.vector.tensor_tensor_scan(
                    kcum[:], k32[:], k32[:], 0.0, Alu.add, Alu.bypass)
                nc.vector.tensor_tensor(q32[:], q32[:], kcum[:], Alu.mult)
                for h in range(2):
                    hp = slice(64 * h, 64 * h + 64)
                    for cc in range(4):
                        cs2 = slice(512 * cc, 512 * (cc + 1))
                        psr = p1ps3.tile([1, 512], FP32, tag="psr",
                                         name=f"psr{h}_{cc}")
                        nc.tensor.matmul(psr[:], onesc[hp], q32[hp, cs2],
                                         start=True, stop=True)
                        nc.scalar.copy(rs_row[64 * h:64 * h + 1, cs2], psr[:])
                # transpose rowsum row into stat cols
                psq = p1ps3.tile([128, 32], FP32, tag="psq", name="psq")
                for h in range(2):
                    for rb in range(16):
                        cl = col_of(rb, h)
                        nc.tensor.transpose(
                            psq[:, cl:cl + 1],
                            rs_row[64 * h:64 * h + 1, 128 * rb:128 * rb + 128],
                            ident[64 * h:64 * h + 1, 64 * h:64 * h + 1])
                nc.vector.tensor_copy(rowsum[:], psq[:])

            # ---------------- P2: scores + eviction ------------------------
            sB_pool = ctx.enter_context(tc.tile_pool(name="sB", bufs=1))
            SB = {}
            for h in range(2):
                for p in range(4):
                    SB[(p, h)] = sB_pool.tile([128, 3200], FP32,
                                              tag=f"sb{p}_{h}", name=f"sb{p}_{h}")

            def emit_scores(pairs, S_of, h, ps_pool):
                units = units_of(pairs, h)
                for (rb, off, p) in units:
                    col = col_of(rb, h)
                    n = 128 * (rb + 1)
                    S = S_of(p, h)
                    rbs = slice(128 * rb, 128 * rb + 128)
                    for ci, c0 in enumerate(range(0, n, 1024)):
                        w = min(1024, n - c0)
                        ps = ps_pool.tile([128, 1024], FP32, tag="sc",
                                          name=f"sc{rb}_{h}_{ci}")
                        for s0 in range(0, w, 512):
                            sw = min(512, w - s0)
                            cs = slice(c0 + s0, c0 + s0 + sw)
                            pss = ps[:, s0:s0 + sw]
                            nc.tensor.matmul(pss, qstack[h][:, rbs],
                                             kdup[h][:, cs],
                                             start=True, stop=False)
                            nc.tensor.matmul(pss, qstack[h][0:64, rbs],
                                             ktl[h][0:64, cs],
                                             start=False, stop=True)
                        last = (c0 + w == n)
                        mainw = w - 128 if last else w
                        m_acc = maxF if ci == 0 else maxG
                        s_acc = sqF if ci == 0 else sqG
                        # sum of squares over FULL chunk (incl diag garbage --
                        # only feeds the sigma estimate)
                        nc.scalar.activation(
                            trash_ev[:, :w], ps[:, :w], Act.Square,
                            bias=0.0, accum_out=s_acc[:, col:col + 1])
                        if mainw > 0:
                            nc.vector.tensor_scalar(
                                out=S[:, off + c0:off + c0 + mainw],
                                in0=ps[:, :mainw],
                                scalar1=0.0, scalar2=MAX_INIT,
                                op0=Alu.add, op1=Alu.max,
                                accum_out=m_acc[:, col:col + 1])
                        if last:
                            nc.vector.tensor_tensor(
                                S[:, off + n - 128:off + n],
                                ps[:, mainw:w], mneg[:], Alu.add)
                            nc.vector.tensor_scalar(
                                out=trash_d[:, :128],
                                in0=S[:, off + n - 128:off + n],
                                scalar1=0.0, scalar2=MAX_INIT,
                                op0=Alu.add, op1=Alu.max,
                                accum_out=maxD[:, col:col + 1])

            with ExitStack() as p2s:
                ps_sc = p2s.enter_context(
                    tc.tile_pool(name="ps_sc", bufs=3, space="PSUM"))
                for h in range(2):
                    emit_scores(B_PAIRS, lambda p, hh: SB[(p, hh)], h, ps_sc)

                sA_pool = ctx.enter_context(tc.tile_pool(name="sA", bufs=1))
                SA = {}
                for h in range(2):
                    for p in range(4):
                        SA[(p, h)] = sA_pool.tile([128, 1152], FP32,
                                                  tag=f"sa{p}_{h}",
                                                  name=f"sa{p}_{h}")
                for h in range(2):
                    emit_scores(A_PAIRS, lambda p, hh: SA[(p, hh)], h, ps_sc)

            # all units: (rb, off, tile)
            all_units = []
            for h in range(2):
                for (rb, off, p) in units_of(B_PAIRS, h):
                    all_units.append((rb, off, h, SB[(p, h)]))
                for (rb, off, p) in units_of(A_PAIRS, h):
                    all_units.append((rb, off, h, SA[(p, h)]))

            # engine split for half passes (balance ~136 blocks each):
            act_half = [(rb, off, h, S) for (rb, off, h, S) in all_units
                        if (rb >= 8) == (h == 0)]
            dve_half = [(rb, off, h, S) for (rb, off, h, S) in all_units
                        if (rb >= 8) != (h == 0)]

            # ---------------- model start ----------------------------------
            nc.vector.tensor_tensor(mx[:], maxF[:], maxG[:], Alu.max)
            nc.vector.tensor_tensor(mx[:], mx[:], maxD[:], Alu.max)
            nc.vector.tensor_tensor(sumsq[:], sqF[:], sqG[:], Alu.add)
            nc.gpsimd.tensor_tensor(tm1[:], rowsum[:], rn[:], Alu.mult)   # mu
            nc.gpsimd.tensor_tensor(tm2[:], sumsq[:], rn2[:], Alu.mult)   # E[s2]
            nc.gpsimd.tensor_tensor(tm3[:], tm1[:], tm1[:], Alu.mult)     # mu^2
            nc.gpsimd.tensor_tensor(tm2[:], tm2[:], tm3[:], Alu.subtract)  # var
            nc.vector.tensor_scalar_max(tm2[:], tm2[:], 1.0e-12)
            nc.scalar.activation(tm3[:], tm2[:], Act.Sqrt, bias=0.0)      # sigma
            nc.gpsimd.tensor_tensor(tm2[:], tm3[:], nvals[:], Alu.mult)   # w
            nc.vector.tensor_scalar_max(tm2[:], tm2[:], 0.3)
            nc.scalar.activation(Fg[:], tm2[:], Act.Ln, bias=0.0)         # u
            # Horner z = poly(u) -> dlt
            nc.gpsimd.tensor_scalar(out=dlt[:], in0=Fg[:], scalar1=ZPOLY[0],
                                    scalar2=ZPOLY[1], op0=Alu.mult, op1=Alu.add)
            for c in ZPOLY[2:]:
                nc.gpsimd.tensor_tensor(dlt[:], dlt[:], Fg[:], Alu.mult)
                nc.gpsimd.tensor_scalar_add(dlt[:], dlt[:], float(c))
            nc.gpsimd.tensor_tensor(dlt[:], dlt[:], tm3[:], Alu.mult)     # sg*z
            nc.gpsimd.tensor_tensor(dlt[:], dlt[:], tm1[:], Alu.add)      # t0
            nc.gpsimd.tensor_scalar_add(tm1[:], mx[:], -1.0)
            nc.vector.tensor_tensor(dlt[:], dlt[:], tm1[:], Alu.max)
            nc.gpsimd.tensor_scalar_add(tm1[:], mx[:], -0.001)
            nc.vector.tensor_tensor(Tt[:], dlt[:], tm1[:], Alu.min)
            nc.gpsimd.tensor_scalar_mul(nT[:], Tt[:], -1.0)

            # ---------------- iterations -----------------------------------
            def f_pass_act(units):
                for (rb, off, h, S) in units:
                    col = col_of(rb, h)
                    n = 128 * (rb + 1)
                    nc.scalar.activation(
                        trash_a[:, :n], S[:, off:off + n], Act.Relu,
                        bias=nT[:, col:col + 1],
                        accum_out=Ft[:, col:col + 1])

            def f_pass_dve(units):
                for (rb, off, h, S) in units:
                    col = col_of(rb, h)
                    n = 128 * (rb + 1)
                    nc.vector.scalar_tensor_tensor(
                        out=trash_c[:, :n], in0=S[:, off:off + n],
                        scalar=nT[:, col:col + 1], in1=zeros_bf[:, :n],
                        op0=Alu.add, op1=Alu.max,
                        accum_out=Ft[:, col:col + 1])

            def c_pass_dve(units):
                for (rb, off, h, S) in units:
                    col = col_of(rb, h)
                    n = 128 * (rb + 1)
                    nc.vector.tensor_scalar(
                        out=trash_c[:, :n], in0=S[:, off:off + n],
                        scalar1=Tt[:, col:col + 1], scalar2=0.0,
                        op0=Alu.is_gt, op1=Alu.add,
                        accum_out=Ct[:, col:col + 1])

            def c_pass_act(units):
                for (rb, off, h, S) in units:
                    col = col_of(rb, h)
                    n = 128 * (rb + 1)
                    nc.scalar.activation(
                        trash_a[:, :n], S[:, off:off + n], Act.Sign,
                        bias=nT[:, col:col + 1],
                        accum_out=Ct[:, col:col + 1])

            # trash tiles reuse score-phase fp16 tiles (same byte footprint)
            trash_a = persist.tile([128, L], BF16, tag="ktl0", name="trash_a")
            trash_c = persist.tile([128, L], BF16, tag="kd0", name="trash_c")

            def taylor_ln():
                # lF = (Fg-1)*(1 - 0.5*(Fg-1))
                nc.gpsimd.tensor_scalar_add(tm1[:], Fg[:], -1.0)
                nc.gpsimd.tensor_scalar(out=tm2[:], in0=tm1[:], scalar1=-0.5,
                                        scalar2=1.0, op0=Alu.mult, op1=Alu.add)
                nc.gpsimd.tensor_tensor(lF[:], tm1[:], tm2[:], Alu.mult)

            for it in range(3):
                f_pass_act(all_units)
                c_pass_dve(all_units)
                nc.vector.tensor_scalar_max(Fg[:], Ft[:], 1.0e-12)
                if it < 2:
                    nc.scalar.activation(lF[:], Fg[:], Act.Ln, bias=0.0)
                else:
                    taylor_ln()
                nc.vector.tensor_scalar_max(Cg[:], Ct[:], 1.0)
                nc.vector.reciprocal(rec[:], Cg[:])
                nc.gpsimd.tensor_tensor(dlt[:], lF[:], Fg[:], Alu.mult)
                nc.gpsimd.tensor_tensor(dlt[:], dlt[:], rec[:], Alu.mult)
                nc.vector.tensor_scalar(out=dlt[:], in0=dlt[:], scalar1=-1.0,
                                        scalar2=1.0, op0=Alu.max, op1=Alu.min)
                nc.gpsimd.tensor_tensor(nT[:], nT[:], dlt[:], Alu.subtract)
                nc.gpsimd.tensor_tensor(Tt[:], Tt[:], dlt[:], Alu.add)

            # stale-slope step (F only, split engines; rec from iter 3)
            f_pass_act(act_half)
            f_pass_dve(dve_half)
            nc.vector.tensor_scalar_max(Fg[:], Ft[:], 1.0e-12)
            taylor_ln()
            nc.gpsimd.tensor_tensor(dlt[:], lF[:], Fg[:], Alu.mult)
            nc.gpsimd.tensor_tensor(dlt[:], dlt[:], rec[:], Alu.mult)
            nc.vector.tensor_scalar(out=dlt[:], in0=dlt[:], scalar1=-1.0,
                                    scalar2=1.0, op0=Alu.max, op1=Alu.min)
            nc.gpsimd.tensor_tensor(nT[:], nT[:], dlt[:], Alu.subtract)
            nc.gpsimd.tensor_tensor(Tt[:], Tt[:], dlt[:], Alu.add)

            # final count (split engines; ACT via Sign trick)
            c_pass_act(act_half)
            c_pass_dve(dve_half)
            # fix ACT cols: C = 0.5*C + wvals  (wvals = 64*(rb+1))
            for sl in (slice(0, 8), slice(24, 32)):
                nc.vector.scalar_tensor_tensor(
                    out=Ct[:, sl], in0=Ct[:, sl], scalar=0.5, in1=wvals[:, sl],
                    op0=Alu.mult, op1=Alu.add)
            nc.vector.tensor_scalar_max(Cg[:], Ct[:], 1.0)
            nc.vector.reciprocal(rec[:], Cg[:])
            nc.gpsimd.tensor_scalar_add(tm1[:], rowsum[:], -1.0)
            nc.gpsimd.tensor_tensor(tau[:], tm1[:], rec[:], Alu.mult)
            nc.gpsimd.tensor_scalar_mul(ntau[:], tau[:], -1.0)

            # ---------------- route -tau into qhx row 64 -------------------
            with ExitStack() as pr:
                ps_r = pr.enter_context(
                    tc.tile_pool(name="ps_r", bufs=2, space="PSUM"))
                st_r = pr.enter_context(tc.tile_pool(name="st_r", bufs=2))
                for sg, base in (("B", 0), ("A", 16)):
                    for h in range(2):
                        sl8 = slice(base + 8 * h, base + 8 * h + 8)
                        xf = st_r.tile([128, 8], FP32, tag=f"xf{h}",
                                       name=f"xf{sg}_{h}")
                        nc.vector.tensor_copy(xf[:], ntau[:, sl8])
                        psx = ps_r.tile([8, 128], FP32, tag="psx",
                                        name=f"psx{sg}_{h}")
                        nc.tensor.transpose(psx[:], xf[:], ident[:])
                        stg = st_r.tile([8, 128], FP32, tag=f"stg{h}",
                                        name=f"stg{sg}_{h}")
                        nc.scalar.copy(stg[:], psx[:])
                        p64 = ps_r.tile([1, 1024], FP32, tag="p64",
                                        name=f"p64{sg}_{h}")
                        for b in range(8):
                            nc.tensor.matmul(
                                p64[:, 128 * b:128 * (b + 1)],
                                ident[0:8, b:b + 1], stg[:],
                                start=True, stop=True)
                        if sg == "B":
                            osl = slice(1024, 2048)
                        else:
                            osl = slice(0, 1024)
                        nc.scalar.copy(qhx[h][64:65, osl], p64[:])

            # ---------------- ST + AV + Wo ---------------------------------
            # PT bins reuse SB tiles (pads to same bytes: 6400 fp16 cols)
            pt_off = {}
            PT = {}
            for b, jts in enumerate(PT_BINS):
                off = 0
                for jt in jts:
                    pt_off[jt] = (b, off)
                    off += (16 - jt) * 128
            for h in range(2):
                for b in range(3):
                    PT[(h, b)] = sB_pool.tile([128, 6400], FP16,
                                              tag=f"sb{b}_{h}",
                                              name=f"pt{h}_{b}")

            outTh = persist.tile([128, L], FP16, tag="qst0", name="outTh")

            with ExitStack() as p3:
                ps_st = p3.enter_context(
                    tc.tile_pool(name="ps_st", bufs=2, space="PSUM"))
                ps_av = p3.enter_context(
                    tc.tile_pool(name="ps_av", bufs=1, space="PSUM"))
                ps_wo = p3.enter_context(
                    tc.tile_pool(name="ps_wo", bufs=1, space="PSUM"))
                ps_z = p3.enter_context(
                    tc.tile_pool(name="ps_z", bufs=1, space="PSUM"))
                wo_pool = p3.enter_context(tc.tile_pool(name="wop", bufs=2))

                def emit_st(h, jt, i_lo, i_hi):
                    """P~^T[j in jt-block, i in [i_lo, i_hi)] into PT bin."""
                    b, off = pt_off[jt]
                    pt_tile = PT[(h, b)]
                    jsl = slice(128 * jt, 128 * jt + 128)
                    for c0 in range(i_lo, i_hi, 512):
                        cw = min(512, i_hi - c0)
                        ps = ps_st.tile([128, 512], FP32, tag="st",
                                        name=f"st{h}_{jt}_{c0}")
                        isl = slice(c0, c0 + cw)
                        nc.tensor.matmul(ps[:, :cw], khx[h][:, jsl],
                                         qhx[h][:, isl], start=True, stop=True)
                        d0 = 128 if c0 == 128 * jt else 0
                        po = off + (c0 - 128 * jt)
                        if d0:
                            nc.vector.scalar_tensor_tensor(
                                out=pt_tile[:, po:po + 128],
                                in0=ps[:, 0:128], scalar=0.0, in1=m01u[:],
                                op0=Alu.max, op1=Alu.mult)
                        if cw > d0:
                            if (jt + h) % 2 == 0:
                                nc.scalar.activation(
                                    pt_tile[:, po + d0:po + cw],
                                    ps[:, d0:cw], Act.Relu, bias=0.0)
                            else:
                                nc.vector.tensor_scalar(
                                    out=pt_tile[:, po + d0:po + cw],
                                    in0=ps[:, d0:cw], scalar1=0.0,
                                    scalar2=0.0, op0=Alu.max, op1=Alu.add)

                def emit_av(h, half):
                    """out^T[d, i] for i in half (0: <1024, 1: >=1024)."""
                    i_lo_h = 1024 * half
                    i_hi_h = 1024 * (half + 1)
                    avp = ps_av.tile([65, 1024], FP32, tag="av",
                                     name=f"av{h}_{half}")
                    jts = [jt for jt in range(16) if 128 * jt < i_hi_h]
                    first = True
                    for jt in jts:
                        b, off = pt_off[jt]
                        lo = max(i_lo_h, 128 * jt)
                        for cc in range(lo, i_hi_h, 512):
                            ce = min(cc + 512, i_hi_h)
                            src = PT[(h, b)][:, off + (cc - 128 * jt):
                                             off + (ce - 128 * jt)]
                            nc.tensor.matmul(
                                avp[:, cc - i_lo_h:ce - i_lo_h],
                                vt[:, jt, 65 * h:65 * h + 65], src,
                                start=first,
                                stop=(jt == jts[-1] and ce == i_hi_h))
                        first = False
                    balanced_copy(outTh[64 * h:64 * h + 64, i_lo_h:i_hi_h],
                                  avp[0:64, :])
                    nc.scalar.copy(zsb[64 * h:64 * h + 1, i_lo_h:i_hi_h],
                                   avp[64:65, :])

                def emit_wo(blk):
                    wo_sb = wo_pool.tile([128, D], FP16, tag="wod",
                                         name=f"wod{blk}")
                    for oc in range(2):
                        osl2 = slice(512 * oc, 512 * (oc + 1))
                        wop0 = ps_wo.tile([128, 512], FP32, tag="wo0",
                                          name=f"wo0_{blk}_{oc}")
                        wop1 = ps_wo.tile([128, 512], FP32, tag="wo1",
                                          name=f"wo1_{blk}_{oc}")
                        nc.tensor.matmul(
                            wop0[:], outTh[0:64, 128 * blk:128 * (blk + 1)],
                            woTh[0:64, osl2], start=True, stop=True)
                        nc.tensor.matmul(
                            wop1[:], outTh[64:128, 128 * blk:128 * (blk + 1)],
                            woTh[64:128, osl2], start=True, stop=True)
                        nc.scalar.activation(
                            wo_sb[:, osl2], wop0[:], Act.Copy,
                            bias=0.0, scale=rzt[:, blk:blk + 1])
                        nc.vector.scalar_tensor_tensor(
                            out=wo_sb[:, osl2], in0=wop1[:],
                            scalar=rzt[:, 16 + blk:16 + blk + 1],
                            in1=wo_sb[:, osl2], op0=Alu.mult, op1=Alu.add)
                    nc.sync.dma_start(
                        out_d.ap()[128 * blk:128 * (blk + 1), :], wo_sb[:])

                # ---- half 1 (i >= 1024): ST_B -> AVh1 -> Z -> wo 8..15
                for h in range(2):
                    for jt in range(16):
                        lo = max(1024, 128 * jt)
                        emit_st(h, jt, lo, 2048)
                for h in range(2):
                    emit_av(h, 1)
                zp1 = ps_z.tile([128, 16], FP32, tag="zp", name="zp1")
                for h in range(2):
                    for b in range(8, 16):
                        nc.tensor.transpose(
                            zp1[:, 8 * h + b - 8:8 * h + b - 7],
                            zsb[64 * h:64 * h + 1, 128 * b:128 * b + 128],
                            ident[64 * h:64 * h + 1, 64 * h:64 * h + 1])
                for h in range(2):
                    dsl = slice(16 * h + 8, 16 * h + 16)
                    ssl = slice(8 * h, 8 * h + 8)
                    nc.vector.tensor_scalar_add(zps_s[:, dsl], zp1[:, ssl],
                                                1.0e-10)
                    nc.vector.reciprocal(rzt[:, dsl], zps_s[:, dsl])
                for blk in range(8, 16):
                    emit_wo(blk)

                # ---- half 0 (i < 1024): ST_A -> AVh0 -> Z -> wo 0..7
                for h in range(2):
                    for jt in range(8):
                        emit_st(h, jt, 128 * jt, 1024)
                for h in range(2):
                    emit_av(h, 0)
                zp0 = ps_z.tile([128, 16], FP32, tag="zp", name="zp0")
                for h in range(2):
                    for b in range(0, 8):
                        nc.tensor.transpose(
                            zp0[:, 8 * h + b:8 * h + b + 1],
                            zsb[64 * h:64 * h + 1, 128 * b:128 * b + 128],
                            ident[64 * h:64 * h + 1, 64 * h:64 * h + 1])
                for h in range(2):
                    dsl = slice(16 * h, 16 * h + 8)
                    ssl = slice(8 * h, 8 * h + 8)
                    nc.vector.tensor_scalar_add(zps_s[:, dsl], zp0[:, ssl],
                                                1.0e-10)
                    nc.vector.reciprocal(rzt[:, dsl], zps_s[:, dsl])
                for blk in range(0, 8):
                    emit_wo(blk)

    nc.compile()
    return nc


_CACHE = {}


def _get_nc():
    if "nc" not in _CACHE:
        _CACHE["nc"] = build_program()
    return _CACHE["nc"]


def _split16(a):
    hi = a.astype(np.float16)
    lo = (a.astype(np.float32) - hi.astype(np.float32)).astype(np.float16)
    return hi, lo


def _host_inputs(x, Wq, Wk, Wv, Wo):
    xT = np.ascontiguousarray(x[0].T).astype(np.float32)
    ii = np.arange(128)
    mneg = np.where(ii[None, :] > ii[:, None], np.float32(NEG_BIG),
                    np.float32(0.0)).astype(np.float32)
    m01u = (ii[None, :] >= ii[:, None]).astype(np.float16)
    ident = np.eye(128, dtype=np.float32)
    # per-unit-column constants [128, 32]
    nvals = np.zeros((128, 32), np.float32)
    rn = np.zeros((128, 32), np.float32)
    rn2 = np.zeros((128, 32), np.float32)
    wvals = np.zeros((128, 32), np.float32)
    r = np.arange(128, dtype=np.float32)
    for h in range(2):
        for rb in range(16):
            cl = col_of(rb, h)
            n = 128 * rb + r + 1.0
            nvals[:, cl] = n
            rn[:, cl] = 1.0 / n
            rn2[:, cl] = 1.0 / (128.0 * (rb + 1))
            wvals[:, cl] = 64.0 * (rb + 1)
    in_maps = []
    for c in range(N_CORES):
        hsl = slice(128 * c, 128 * (c + 1))
        wqh, wql = _split16((Wq[hsl] * np.float32(SCALE)).T)
        wkh, wkl = _split16(Wk[hsl].T)
        wvh, _ = _split16(Wv[hsl].T)
        in_maps.append({
            "xT": xT,
            "wqh": np.ascontiguousarray(wqh),
            "wql": np.ascontiguousarray(wql),
            "wkh": np.ascontiguousarray(wkh),
            "wkl": np.ascontiguousarray(wkl),
            "wvh": np.ascontiguousarray(wvh),
            "woT": np.ascontiguousarray(Wo[:, hsl].T).astype(np.float16),
            "mneg": mneg,
            "m01u": m01u,
            "ident": ident,
            "nvals": nvals,
            "rn": rn,
            "rn2": rn2,
            "wvals": wvals,
        })
    return in_maps


def kernel(x, Wq, Wk, Wv, Wo, _trace=False, _debug=None):
    nc = _get_nc()
    in_maps = _host_inputs(np.asarray(x), np.asarray(Wq), np.asarray(Wk),
                           np.asarray(Wv), np.asarray(Wo))
    res = run_bass_kernel_spmd(nc, in_maps, core_ids=list(range(N_CORES)),
                               trace=_trace)
    out = np.zeros((L, D), np.float32)
    for c in range(N_CORES):
        out += res.results[c]["out"].astype(np.float32)
    if _trace:
        _CACHE["last_results"] = res
    return out.reshape(1, L, D)
